# revision 39
# baseline (speedup 1.0000x reference)
"""BertBlock kernel for 8 Trainium2 NeuronCores.

Sharding: pure data-parallel over (batch, half-sequence) tokens: core c
handles batch element c//2, query-token half c%2 (1024 tokens). Each core
recomputes K/V for the full 2048-token sequence of its batch element (the
duplicated K/V projection work is far cheaper than any 2-rank collective),
so no collectives are needed at all.

Device layout is feature-major ([feature, token]) end to end. The large
projections (QKV / O / MLP) run in fp8e4m3 with DoubleRow perf mode (two
128-deep contraction tiles per PE pass); weights are pre-scaled by 64 on
the host so they sit in fp8's normal range, and the 1/64 descale is folded
into the PSUM-drain ops. Scores and (optionally) AV stay bf16. Softmax
denominators come from an extra ones-column in the attention-V stationary
operand; the per-head divide uses a fast-approx DVE reciprocal and a
GPSIMD partition-broadcast so the Act engine runs exp back-to-back and
the PE never blocks on normalization.
"""

import numpy as np
import ml_dtypes

P = 128
B = 4
S = 2048          # sequence length (keys)
SQ = 1024         # query tokens per core
H = 768
HC = H // P       # 6 feature chunks
NH = 12
DH = 64
FF = 3072
FC = FF // P      # 24
TS = S // P       # 16 key-token chunks
TQ = SQ // P      # 8 query-token chunks
N_CORES = 8
EPS = 1e-5
BF16 = ml_dtypes.bfloat16
F8 = ml_dtypes.float8_e4m3
WSCALE = 64.0     # host-side weight pre-scale for fp8

# fp8 toggles per matmul group
FP8_QKV = True
FP8_O = True
FP8_MLP1 = True
FP8_MLP2 = False   # W2/h quantization is the largest rel-err contributor
FP8_AV = True

DEBUG_DUMPS = False  # adds intermediate-tensor outputs for debugging

_CACHE = {}


def _emit(nc, tc, t, mybir, make_identity):
    """Emit the per-core program. `t` maps tensor name -> DRAM AP."""
    from contextlib import ExitStack

    f32 = mybir.dt.float32
    f32r = mybir.dt.float32r
    bf16 = mybir.dt.bfloat16
    fp8 = mybir.dt.float8e4
    AF = mybir.ActivationFunctionType
    OP = mybir.AluOpType
    DR = mybir.MatmulPerfMode.DoubleRow

    def mm(ps, lhsT, rhs, start, stop, perf_mode=None):
        nc.tensor.matmul(ps, lhsT=lhsT, rhs=rhs, start=start, stop=stop,
                         perf_mode=perf_mode)

    with ExitStack() as ctx:
        aux = ctx.enter_context(tc.tile_pool(name="aux", bufs=1))

        def aux_load(name, shape, dtype=f32, eng=None):
            tl = aux.tile(shape, dtype, tag=name)
            (eng or nc.gpsimd).dma_start(tl[:], t[name])
            return tl

        bq_s = aux_load("bq2", [P, HC])
        bk_s = aux_load("bk2", [P, HC])
        bo_s = aux_load("bo2", [P, HC])
        b2_s = aux_load("b22", [P, HC])
        l1w_s = aux_load("l1w", [P, HC])
        l1b_s = aux_load("l1b", [P, HC])
        l2w_s = aux_load("l2w", [P, HC])
        l2b_s = aux_load("l2b", [P, HC])
        b1_s = aux_load("b12", [P, FC])
        bvb_s = aux.tile([P, H], bf16)
        nc.gpsimd.dma_start(bvb_s[:], t["bv"].partition_broadcast(P))
        ones_f = aux.tile([P, 1], f32)
        nc.vector.memset(ones_f[:], 1.0)
        ones_s = aux.tile([P, 1], f32r)
        nc.vector.tensor_copy(ones_s[:], ones_f[:])
        ones_b = aux.tile([P, 1], bf16)
        nc.vector.memset(ones_b[:], 1.0)
        ident_b = aux.tile([P, P], bf16)
        make_identity(nc, ident_b[:])

        qkv_dt = fp8 if FP8_QKV else bf16
        o_dt = fp8 if FP8_O else bf16
        m1_dt = fp8 if FP8_MLP1 else bf16
        m2_dt = fp8 if FP8_MLP2 else bf16
        av_dt = fp8 if FP8_AV else bf16
        qkv_sc = 1.0 / WSCALE if FP8_QKV else 1.0
        o_sc = 1.0 / WSCALE if FP8_O else 1.0
        m1_sc = 1.0 / WSCALE if FP8_MLP1 else 1.0
        m2_sc = 1.0 / WSCALE if FP8_MLP2 else 1.0

        # x1 (LN1 output) outlives the attention scopes below. bf16 is
        # plenty for the MLP residual add.
        keep = ctx.enter_context(tc.tile_pool(name="keep", bufs=1))
        x1_s = keep.tile([P, HC, SQ], bf16)
        x18_s = keep.tile([P, HC, SQ], m1_dt)
        # Wo and W1 are fully preloaded during attention (the DMA queues
        # are idle there) so the O-projection and MLP1 never stall on
        # weight streaming.
        w1p = ctx.enter_context(tc.tile_pool(name="w1_pre", bufs=1))
        w1all = w1p.tile([P, HC, FF], m1_dt)

        def proj_accum(pp, w_t, rhs_tile, rhs_lo, ps, fp8_on, n_slices,
                       tag=None):
            """Accumulate a full-contraction projection into psum `ps`.

            w_t: [P, HC, M] stationary; rhs_tile[:, kc, rhs_lo + n*512 ...]
            moving. n_slices: list of (off, width) output slices.
            """
            if fp8_on:
                for i, (off, wd) in enumerate(n_slices):
                    for kc2 in range(HC // 2):
                        mm(
                            ps[:, off : off + wd],
                            w_t[:, 2 * kc2 : 2 * kc2 + 2, :],
                            rhs_tile[
                                :, 2 * kc2 : 2 * kc2 + 2,
                                rhs_lo + off : rhs_lo + off + wd,
                            ],
                            kc2 == 0,
                            kc2 == HC // 2 - 1,
                            perf_mode=DR,
                        )
            else:
                for i, (off, wd) in enumerate(n_slices):
                    for kc in range(HC):
                        mm(
                            ps[:, off : off + wd],
                            w_t[:, kc, :],
                            rhs_tile[
                                :, kc, rhs_lo + off : rhs_lo + off + wd
                            ],
                            kc == 0,
                            kc == HC - 1,
                        )

        with tc.tile_pool(name="resid", bufs=1) as resid:
            # bf16 residual copy of this core's query tokens
            xq_s = resid.tile([P, HC, SQ], bf16)
            woall = resid.tile([P, HC, H], o_dt)
            qs = (nc.sync, nc.scalar, nc.gpsimd)
            for j in range(HC):
                qs[j % 3].dma_start(
                    xq_s[:, j, :],
                    t["xq"].rearrange("(c p) s -> p c s", p=P)[:, j, :],
                )
            with tc.tile_pool(name="attn_out", bufs=1) as aop:
                attnT_s = aop.tile([P, HC, SQ], o_dt)

                with tc.tile_pool(name="qkv_keep", bufs=1) as p2:
                    # qTz[p, h, q]: head h's 64 q-rows live at partitions
                    # (h%2)*64..+64 of plane h; the other 64 partitions stay
                    # zero so scores can contract over all 128 partitions.
                    qTz_s = p2.tile([P, NH, SQ], bf16)
                    nc.gpsimd.memset(qTz_s[:], 0.0)
                    kT_s = p2.tile([P, HC, S], bf16)
                    # v_s[p, kt, h*VS .. h*VS+64] = V rows for head h,
                    # col h*VS+64 = ones (softmax denominator); zero pad up
                    # to VS and at the tail lets every head take a full
                    # 128-col stationary slice v_s[:, kt, h*VS : h*VS+128].
                    # VS=128 in fp8 mode: dual-fp8 ldweights requires the
                    # kt plane stride (and safest, the per-head offsets) to
                    # be multiples of 128, so each head gets a private
                    # 128-col window.
                    VS = 128 if FP8_AV else 65
                    vcols = (NH - 1) * VS + P
                    v_s = p2.tile([P, TS, vcols], av_dt)
                    v_view = v_s[:, :, 0 : NH * VS].rearrange(
                        "p t (h d) -> p t h d", h=NH
                    )
                    nc.vector.memset(v_view[:, :, :, DH : DH + 1], 1.0)
                    if VS > DH + 1:
                        nc.gpsimd.memset(v_view[:, :, :, DH + 1 :], 0.0)
                    if vcols > NH * VS:
                        nc.gpsimd.memset(v_s[:, :, NH * VS :], 0.0)

                    # ---------------- QKV projections ----------------
                    with tc.tile_pool(name="qkvph", bufs=1) as ph, tc.tile_pool(
                        name="wstream", bufs=3
                    ) as ws, tc.tile_pool(
                        name="qkv_ps", bufs=3, space="PSUM"
                    ) as pp:
                        # full-sequence x in compute dtype, 6 chunk DMAs
                        xT_s = ph.tile([P, HC, S], qkv_dt)
                        for j in range(HC):
                            qs[(j + 2) % 3].dma_start(
                                xT_s[:, j, :],
                                t["xT"].rearrange("(c p) s -> p c s", p=P)[
                                    :, j, :
                                ],
                            )

                        # Q (this core's 1024 query tokens). Reads the bf16
                        # per-core query slice: the SPMD program is shared
                        # across cores, so Q cannot read xT at a per-core
                        # offset. bf16 Q also keeps the softmax logits clean.
                        for j in range(HC):
                            w_t = ws.tile([P, HC, P], bf16, tag="wq")
                            qs[j % 3].dma_start(
                                w_t[:],
                                t["Wq"][:, j * P : (j + 1) * P].rearrange(
                                    "(c p) m -> p c m", p=P
                                ),
                            )
                            ps = pp.tile([P, SQ], f32, tag="qkps")
                            proj_accum(
                                pp, w_t, xq_s, 0, ps, False,
                                [(0, 512), (512, 512)],
                            )
                            nc.vector.tensor_scalar(
                                qTz_s[0:DH, 2 * j, :], ps[0:DH, :],
                                1.0, bq_s[0:DH, j : j + 1],
                                OP.mult, OP.add,
                            )
                            nc.vector.tensor_scalar(
                                qTz_s[DH:P, 2 * j + 1, :], ps[DH:P, :],
                                1.0, bq_s[DH:P, j : j + 1],
                                OP.mult, OP.add,
                            )

                        # K (all 2048 tokens)
                        for j in range(HC):
                            wk_t = ws.tile([P, HC, P], qkv_dt, tag="w")
                            qs[(j + 1) % 3].dma_start(
                                wk_t[:],
                                t["Wk"][:, j * P : (j + 1) * P].rearrange(
                                    "(c p) m -> p c m", p=P
                                ),
                            )
                            for hf in range(2):
                                ps = pp.tile([P, SQ], f32, tag="qkps")
                                proj_accum(
                                    pp, wk_t, xT_s, hf * SQ, ps, FP8_QKV,
                                    [(0, 512), (512, 512)],
                                )
                                nc.vector.tensor_scalar(
                                    kT_s[:, j, hf * SQ : (hf + 1) * SQ],
                                    ps[:], qkv_sc, bk_s[:, j : j + 1],
                                    OP.mult, OP.add,
                                )

                        # V (token-major with per-head ones column)
                        wv_t = ws.tile([P, HC, H], qkv_dt, tag="wv", bufs=1)
                        wv_r = t["Wv"].rearrange("(c p) m -> p c m", p=P)
                        nc.sync.dma_start(wv_t[:, 0:3, :], wv_r[:, 0:3, :])
                        nc.scalar.dma_start(wv_t[:, 3:6, :], wv_r[:, 3:6, :])
                        # Preload O / MLP1 weights now: these transfer during
                        # the long exp-bound attention phase on idle queues.
                        nc.gpsimd.dma_start(
                            woall[:], t["Wo"].rearrange("(c p) m -> p c m", p=P)
                        )
                        w1_r = t["W1"].rearrange("(c p) n -> p c n", p=P)
                        nc.sync.dma_start(
                            w1all[:, :, 0 : FF // 2], w1_r[:, :, 0 : FF // 2]
                        )
                        nc.scalar.dma_start(
                            w1all[:, :, FF // 2 :], w1_r[:, :, FF // 2 :]
                        )
                        for tt in range(TS):
                            ps = pp.tile([P, SQ], f32, tag="qkps")
                            if FP8_QKV:
                                for kc2 in range(HC // 2):
                                    for off, wd in ((0, 512), (512, 256)):
                                        mm(
                                            ps[:, off : off + wd],
                                            xT_s[
                                                :, 2 * kc2 : 2 * kc2 + 2,
                                                tt * P : (tt + 1) * P,
                                            ],
                                            wv_t[
                                                :, 2 * kc2 : 2 * kc2 + 2,
                                                off : off + wd,
                                            ],
                                            kc2 == 0,
                                            kc2 == HC // 2 - 1,
                                            perf_mode=DR,
                                        )
                            else:
                                for kc in range(HC):
                                    for off, wd in ((0, 512), (512, 256)):
                                        mm(
                                            ps[:, off : off + wd],
                                            xT_s[:, kc, tt * P : (tt + 1) * P],
                                            wv_t[:, kc, off : off + wd],
                                            kc == 0,
                                            kc == HC - 1,
                                        )
                            nc.vector.scalar_tensor_tensor(
                                out=v_view[:, tt, :, 0:DH],
                                in0=ps[:, 0:H].rearrange("p (h d) -> p h d", h=NH),
                                scalar=qkv_sc,
                                in1=bvb_s[:].rearrange("p (h d) -> p h d", h=NH),
                                op0=OP.mult,
                                op1=OP.add,
                            )

                    if DEBUG_DUMPS:
                        nc.sync.dma_start(t["dq"], qTz_s[:])
                        nc.sync.dma_start(t["dk"], kT_s[:])
                        nc.sync.dma_start(t["dv"], v_s[:])

                    # ---------------- attention ----------------
                    with tc.tile_pool(name="attn_sb", bufs=1) as ab, tc.tile_pool(
                        name="probs", bufs=3
                    ) as prp, tc.tile_pool(
                        name="sc_ps", bufs=2, space="PSUM"
                    ) as pps, tc.tile_pool(
                        name="av_ps", bufs=2, space="PSUM"
                    ) as ppa:
                        avs = {}
                        spills = {}

                        def spill_head(h):
                            # Raw accumulator (attn rows) plus the sums row
                            # straight to SBUF on the DVE so the psum slot
                            # frees fast and Act stays exp-only. The sums row
                            # lands on partition 0: the fast-reciprocal
                            # custom DVE op cannot shift partitions.
                            av = avs.pop(h)
                            raw = ab.tile([DH, SQ], f32, tag="raw", bufs=3)
                            nc.vector.tensor_copy(raw[:], av[0:DH, :])
                            sums = ab.tile([1, SQ], f32, tag="sums", bufs=3)
                            nc.vector.tensor_copy(sums[:], av[DH : DH + 1, :])
                            spills[h] = (raw, sums)

                        def normalize_head(h):
                            """Divide head h's attention rows by the softmax
                            sums and place them into attnT.  Emitted one head
                            behind the matmul stream; touches no PSUM so the
                            PE never waits on it."""
                            hc = h // 2
                            raw, sums = spills.pop(h)
                            rec = ab.tile([1, SQ], f32, tag="rec", bufs=2)
                            with nc.allow_low_precision(
                                reason="softmax denominators are O(100) and "
                                "smooth; 18-bit reciprocal is plenty"
                            ):
                                nc.vector.reciprocal_approx_fast(
                                    out=rec[:], in_=sums[:]
                                )
                            bc = ab.tile([DH, SQ], f32, tag="bc", bufs=2)
                            nc.gpsimd.partition_broadcast(bc[:], rec[:])
                            if h % 2 == 0:
                                nc.vector.tensor_tensor(
                                    attnT_s[0:DH, hc, :], raw[0:DH, :],
                                    bc[:], OP.mult,
                                )
                            else:
                                tmp = ab.tile([DH, SQ], o_dt, tag="tmp", bufs=2)
                                nc.vector.tensor_tensor(
                                    tmp[:], raw[0:DH, :], bc[:], OP.mult
                                )
                                nc.sync.dma_start(
                                    attnT_s[DH:P, hc, :], tmp[:]
                                )

                        def emit_av(h, av, ktp, pr):
                            # pr: [P, 2, SQ] exp tile pair (kt = 2*ktp, +1)
                            if FP8_AV:
                                for n in range(2):
                                    mm(
                                        av[:, n * 512 : (n + 1) * 512],
                                        v_s[
                                            :,
                                            2 * ktp : 2 * ktp + 2,
                                            h * VS : h * VS + P,
                                        ],
                                        pr[:, :, n * 512 : (n + 1) * 512],
                                        ktp == 0,
                                        ktp == TS // 2 - 1,
                                        perf_mode=DR,
                                    )
                            else:
                                for i in range(2):
                                    for n in range(2):
                                        mm(
                                            av[:, n * 512 : (n + 1) * 512],
                                            v_s[
                                                :,
                                                2 * ktp + i,
                                                h * VS : h * VS + P,
                                            ],
                                            pr[:, i, n * 512 : (n + 1) * 512],
                                            ktp == 0 and i == 0,
                                            ktp == TS // 2 - 1 and i == 1,
                                        )

                        for h in range(NH):
                            hc = h // 2
                            av = ppa.tile([P, SQ], f32, tag="av")
                            avs[h] = av
                            pending = []
                            for ktp in range(TS // 2):
                                pr = prp.tile([P, 2, SQ], av_dt, tag="pr")
                                for i in range(2):
                                    kt = 2 * ktp + i
                                    sc = pps.tile([P, SQ], f32, tag="sc")
                                    lhsT_k = kT_s[
                                        :, hc, kt * P : (kt + 1) * P
                                    ]
                                    for n in range(2):
                                        mm(
                                            sc[:, n * 512 : (n + 1) * 512],
                                            lhsT_k,
                                            qTz_s[
                                                :, h, n * 512 : (n + 1) * 512
                                            ],
                                            True,
                                            True,
                                        )
                                    nc.scalar.activation(
                                        pr[:, i, :], sc[:], AF.Exp,
                                        scale=0.125,
                                    )
                                pending.append((ktp, pr))
                                if len(pending) > 1:
                                    emit_av(h, av, *pending.pop(0))
                            for p_ in pending:
                                emit_av(h, av, *p_)
                            spill_head(h)
                            if h > 0:
                                normalize_head(h - 1)
                        normalize_head(NH - 1)

                if DEBUG_DUMPS:
                    nc.sync.dma_start(t["dattn"], attnT_s[:])

                # ---------------- O-projection + residual + LN1 ----------------
                with tc.tile_pool(name="oproj", bufs=1) as op_, tc.tile_pool(
                    name="o_ps", bufs=2, space="PSUM"
                ) as ppo, tc.tile_pool(
                    name="st_ps", bufs=1, space="PSUM"
                ) as ppst:
                    r1_s = op_.tile([P, HC, SQ], f32r)
                    sum_ps = ppst.tile([1, SQ], f32, tag="lnsum", bufs=1)
                    sq_ps = ppst.tile([1, SQ], f32, tag="lnsq", bufs=1)
                    for j in range(HC):
                        ps = ppo.tile([P, SQ], f32, tag="ops")
                        proj_accum(
                            ppo, woall[:, :, j * P : (j + 1) * P],
                            attnT_s, 0, ps, FP8_O,
                            [(0, 512), (512, 512)],
                        )
                        to = op_.tile([P, SQ], f32, tag="to", bufs=2)
                        nc.scalar.activation(
                            to[:], ps[:], AF.Identity,
                            bias=bo_s[:, j : j + 1], scale=o_sc,
                        )
                        nc.vector.tensor_tensor(
                            r1_s[:, j, :], to[:], xq_s[:, j, :], OP.add
                        )
                        sq_t = op_.tile([P, SQ], f32r, tag="sqt", bufs=2)
                        nc.vector.tensor_tensor(
                            sq_t[:], r1_s[:, j, :], r1_s[:, j, :], OP.mult
                        )
                        for n in range(2):
                            mm(
                                sum_ps[:, n * 512 : (n + 1) * 512],
                                ones_s[:],
                                r1_s[:, j, n * 512 : (n + 1) * 512],
                                j == 0, j == HC - 1,
                            )
                            mm(
                                sq_ps[:, n * 512 : (n + 1) * 512],
                                ones_s[:],
                                sq_t[:, n * 512 : (n + 1) * 512],
                                j == 0, j == HC - 1,
                            )
                    if DEBUG_DUMPS:
                        nc.sync.dma_start(
                            t["dr1"], r1_s[:].bitcast(f32)
                        )
                    _ln_normalize(
                        nc, tc, mybir, op_, sum_ps, sq_ps, r1_s,
                        [x1_s, x18_s], l1w_s, l1b_s,
                    )
                    if DEBUG_DUMPS:
                        nc.sync.dma_start(t["dx1"], x1_s[:])

        # ---------------- MLP + LN2 + output ----------------
        with tc.tile_pool(name="mlp", bufs=1) as mp, tc.tile_pool(
            name="st2_ps", bufs=1, space="PSUM"
        ) as ppst2:
            hT_s = mp.tile([P, FC, SQ], m2_dt)
            r2_s = mp.tile([P, HC, SQ], bf16)
            # W2 in two halves so MLP2's first chunks don't wait for the
            # whole tensor; both transfer during the LN1 tail + MLP1.
            w2_s = mp.tile([P, FC, H], m2_dt)
            w2_r = t["W2"].rearrange("(c p) m -> p c m", p=P)
            nc.sync.dma_start(w2_s[:, 0 : FC // 2, :], w2_r[:, 0 : FC // 2, :])
            nc.scalar.dma_start(w2_s[:, FC // 2 :, :], w2_r[:, FC // 2 :, :])
            sum2_ps = ppst2.tile([1, SQ], f32, tag="ln2sum", bufs=1)
            sq2_ps = ppst2.tile([1, SQ], f32, tag="ln2sq", bufs=1)
            with tc.tile_pool(name="m_ps", bufs=2, space="PSUM") as ppm:
                for m in range(FC):
                    ps = ppm.tile([P, SQ], f32, tag="mps")
                    proj_accum(
                        ppm, w1all[:, :, m * P : (m + 1) * P], x18_s, 0,
                        ps, FP8_MLP1, [(0, 512), (512, 512)],
                    )
                    nc.scalar.activation(
                        hT_s[:, m, :], ps[:], AF.Gelu,
                        bias=b1_s[:, m : m + 1], scale=m1_sc,
                    )

                for j in range(HC):
                    ps = ppm.tile([P, SQ], f32, tag="mps")
                    if FP8_MLP2:
                        for n in range(2):
                            for kc2 in range(FC // 2):
                                mm(
                                    ps[:, n * 512 : (n + 1) * 512],
                                    w2_s[
                                        :, 2 * kc2 : 2 * kc2 + 2,
                                        j * P : (j + 1) * P,
                                    ],
                                    hT_s[
                                        :, 2 * kc2 : 2 * kc2 + 2,
                                        n * 512 : (n + 1) * 512,
                                    ],
                                    kc2 == 0,
                                    kc2 == FC // 2 - 1,
                                    perf_mode=DR,
                                )
                    else:
                        for n in range(2):
                            for kc in range(FC):
                                mm(
                                    ps[:, n * 512 : (n + 1) * 512],
                                    w2_s[:, kc, j * P : (j + 1) * P],
                                    hT_s[:, kc, n * 512 : (n + 1) * 512],
                                    kc == 0,
                                    kc == FC - 1,
                                )
                    to = mp.tile([P, SQ], f32, tag="to2", bufs=1)
                    nc.scalar.activation(
                        to[:], ps[:], AF.Identity,
                        bias=b2_s[:, j : j + 1], scale=m2_sc,
                    )
                    nc.vector.tensor_tensor(
                        r2_s[:, j, :], to[:], x1_s[:, j, :], OP.add
                    )
                    sq_t = mp.tile([P, SQ], bf16, tag="sqt2", bufs=2)
                    nc.vector.tensor_tensor(
                        sq_t[:], r2_s[:, j, :], r2_s[:, j, :], OP.mult
                    )
                    for n in range(2):
                        mm(
                            sum2_ps[:, n * 512 : (n + 1) * 512],
                            ones_b[:],
                            r2_s[:, j, n * 512 : (n + 1) * 512],
                            j == 0, j == HC - 1,
                        )
                        mm(
                            sq2_ps[:, n * 512 : (n + 1) * 512],
                            ones_b[:],
                            sq_t[:, n * 512 : (n + 1) * 512],
                            j == 0, j == HC - 1,
                        )
                if DEBUG_DUMPS:
                    nc.sync.dma_start(t["dh"], hT_s[:])
                    nc.sync.dma_start(t["dr2"], r2_s[:])

            # LN2 normalizes r2 in place; each chunk is transposed to
            # token-major as soon as it is normalized, and each token tile
            # is stored as soon as its last chunk lands.
            with tc.tile_pool(name="outp", bufs=1) as outp, tc.tile_pool(
                name="tr_ps", bufs=4, space="PSUM"
            ) as ppt:
                out_all = outp.tile([P, TQ, H], bf16)

                def ln2_chunk(j):
                    for tt in range(TQ):
                        tps = ppt.tile([P, P], bf16, tag="tr")
                        nc.tensor.transpose(
                            tps[:],
                            r2_s[:, j, tt * P : (tt + 1) * P],
                            ident_b[:],
                        )
                        if tt % 2 == 0:
                            nc.scalar.activation(
                                out_all[:, tt, j * P : (j + 1) * P], tps[:],
                                AF.Identity,
                            )
                        else:
                            nc.vector.tensor_copy(
                                out_all[:, tt, j * P : (j + 1) * P], tps[:]
                            )

                _ln_normalize(
                    nc, tc, mybir, mp, sum2_ps, sq2_ps, r2_s, [r2_s],
                    l2w_s, l2b_s, chunk_cb=ln2_chunk,
                )
                for tt in range(TQ):
                    qs[tt % 3].dma_start(
                        t["y"][tt * P : (tt + 1) * P, :], out_all[:, tt, :]
                    )


def _ln_normalize(nc, tc, mybir, pool, sum_ps, sq_ps, src_s, dsts, w_s, b_s,
                  chunk_cb=None):
    """Feature-major LayerNorm given accumulated sum / sum-of-squares rows.

    src_s: [P, HC, SQ]. Stats ops run on [1, SQ] rows; mean/rstd broadcast
    across partitions on the GPSIMD engine; the two tensor-tensor steps run
    on the DVE; the final scale-bias (with dtype cast) runs on the Act
    engine once per entry in `dsts` (each a [P, HC, SQ] AP). chunk_cb(j)
    is called after chunk j's outputs are written (for pipelining)."""
    f32 = mybir.dt.float32
    AF = mybir.ActivationFunctionType
    OP = mybir.AluOpType

    f32r = mybir.dt.float32r

    def fm_slice(j):
        sl = src_s[:, j, :]
        return sl.bitcast(f32) if src_s.dtype == f32r else sl

    # Stats chain spread across engines: mean on Act, its broadcast on
    # GPSIMD immediately after, the variance/reciprocal smalls on DVE,
    # rstd sqrt back on Act, its broadcast on GPSIMD. The (src - mean)
    # step for half the chunks runs on GPSIMD between the two
    # broadcasts, in parallel with the DVE smalls.
    mean = pool.tile([1, SQ], f32, tag="lnmean", bufs=1)
    nc.scalar.activation(mean[:], sum_ps[:], AF.Identity, scale=1.0 / H)
    mb = pool.tile([P, SQ], f32, tag="lnmb", bufs=1)
    nc.gpsimd.partition_broadcast(mb[:], mean[:])
    m2 = pool.tile([1, SQ], f32, tag="lnsm", bufs=2)
    nc.vector.tensor_tensor(m2[:], mean[:], mean[:], OP.mult)
    vpe = pool.tile([1, SQ], f32, tag="lnsm", bufs=2)
    nc.vector.scalar_tensor_tensor(
        out=vpe[:], in0=sq_ps[:], scalar=1.0 / H, in1=m2[:], op0=OP.mult,
        op1=OP.subtract,
    )
    nc.vector.tensor_scalar_add(vpe[:], vpe[:], EPS)
    rvar = pool.tile([1, SQ], f32, tag="lnsm", bufs=2)
    with nc.allow_low_precision(reason="18-bit 1/(var+eps) is benign"):
        nc.vector.reciprocal_approx_fast(out=rvar[:], in_=vpe[:])
    # (src - mean) for the back chunks runs on GPSIMD while the DVE works
    # through the variance smalls above.
    t1s = {}
    for j in range(HC // 2, HC):
        t1 = pool.tile([P, SQ], f32, tag=f"lnt1g{j}", bufs=1)
        nc.gpsimd.tensor_tensor(t1[:], fm_slice(j), mb[:], OP.subtract)
        t1s[j] = t1
    rstd = pool.tile([1, SQ], f32, tag="lnrstd", bufs=1)
    nc.scalar.activation(rstd[:], rvar[:], AF.Sqrt)
    rb = pool.tile([P, SQ], f32, tag="lnrb", bufs=1)
    nc.gpsimd.partition_broadcast(rb[:], rstd[:])
    for j in range(HC):
        if j < HC // 2:
            t1 = pool.tile([P, SQ], f32, tag="lnt1v", bufs=1)
            nc.vector.tensor_tensor(t1[:], fm_slice(j), mb[:], OP.subtract)
        else:
            t1 = t1s.pop(j)
        t2 = pool.tile([P, SQ], f32, tag="lnt2", bufs=2)
        eng = nc.vector if j < 4 else nc.gpsimd
        eng.tensor_tensor(t2[:], t1[:], rb[:], OP.mult)
        for dst in dsts:
            nc.scalar.activation(
                dst[:, j, :], t2[:], AF.Identity,
                bias=b_s[:, j : j + 1], scale=w_s[:, j : j + 1],
            )
        if chunk_cb is not None:
            chunk_cb(j)


def _build():
    import concourse.bacc as bacc
    import concourse.tile as tile
    import concourse.mybir as mybir
    from concourse.masks import make_identity

    f32 = mybir.dt.float32
    bf16 = mybir.dt.bfloat16
    fp8 = mybir.dt.float8e4

    qkv_dt = fp8 if FP8_QKV else bf16
    o_dt = fp8 if FP8_O else bf16
    m1_dt = fp8 if FP8_MLP1 else bf16
    m2_dt = fp8 if FP8_MLP2 else bf16

    nc = bacc.Bacc(
        "TRN2", target_bir_lowering=False, debug=False, num_devices=N_CORES
    )
    specs = [
        ("xT", [H, S], qkv_dt, "ExternalInput"),
        ("xq", [H, SQ], bf16, "ExternalInput"),
        ("Wq", [H, H], bf16, "ExternalInput"),
        ("Wk", [H, H], qkv_dt, "ExternalInput"),
        ("Wv", [H, H], qkv_dt, "ExternalInput"),
        ("Wo", [H, H], o_dt, "ExternalInput"),
        ("W1", [H, FF], m1_dt, "ExternalInput"),
        ("W2", [FF, H], m2_dt, "ExternalInput"),
        ("bq2", [P, HC], f32, "ExternalInput"),
        ("bk2", [P, HC], f32, "ExternalInput"),
        ("bv", [H], f32, "ExternalInput"),
        ("bo2", [P, HC], f32, "ExternalInput"),
        ("b12", [P, FC], f32, "ExternalInput"),
        ("b22", [P, HC], f32, "ExternalInput"),
        ("l1w", [P, HC], f32, "ExternalInput"),
        ("l1b", [P, HC], f32, "ExternalInput"),
        ("l2w", [P, HC], f32, "ExternalInput"),
        ("l2b", [P, HC], f32, "ExternalInput"),
        ("y", [SQ, H], bf16, "ExternalOutput"),
    ]
    if DEBUG_DUMPS:
        av_dt = fp8 if FP8_AV else bf16
        vs = 128 if FP8_AV else 65
        vcols = (12 - 1) * vs + P
        specs += [
            ("dq", [P, NH, SQ], bf16, "ExternalOutput"),
            ("dk", [P, HC, S], bf16, "ExternalOutput"),
            ("dv", [P, 16, vcols], av_dt, "ExternalOutput"),
            ("dattn", [P, HC, SQ], o_dt, "ExternalOutput"),
            ("dr1", [P, HC, SQ], f32, "ExternalOutput"),
            ("dx1", [P, HC, SQ], bf16, "ExternalOutput"),
            ("dh", [P, FC, SQ], m2_dt, "ExternalOutput"),
            ("dr2", [P, HC, SQ], bf16, "ExternalOutput"),
        ]
    t = {
        name: nc.dram_tensor(name, shape, dt, kind=kind).ap()
        for name, shape, dt, kind in specs
    }
    with tile.TileContext(nc) as tc:
        _emit(nc, tc, t, mybir, make_identity)
    nc.compile()
    return nc


def _chunk_major(v):
    """[C*P] -> [P, C] with entry [p, c] = v[c*P + p]."""
    return np.ascontiguousarray(v.reshape(-1, P).T)


def prepare_in_maps(inputs):
    inp = {k: np.asarray(v) for k, v in inputs.items()}
    x = inp["x"].astype(np.float32)

    def wcast(w, on):
        w = w.astype(np.float32)
        if on:
            return (w * WSCALE).astype(F8)
        return w.astype(BF16)

    shared = {
        "Wq": inp["Wq"].astype(np.float32).astype(BF16),
        "Wk": wcast(inp["Wk"], FP8_QKV),
        "Wv": wcast(inp["Wv"], FP8_QKV),
        "Wo": wcast(inp["Wo"], FP8_O),
        "W1": wcast(inp["W1"], FP8_MLP1),
        "W2": wcast(inp["W2"], FP8_MLP2),
        "bq2": _chunk_major(inp["bq"].astype(np.float32)),
        "bk2": _chunk_major(inp["bk"].astype(np.float32)),
        "bv": inp["bv"].astype(np.float32),
        "bo2": _chunk_major(inp["bo"].astype(np.float32)),
        "b12": _chunk_major(inp["b1"].astype(np.float32)),
        "b22": _chunk_major(inp["b2"].astype(np.float32)),
        "l1w": _chunk_major(inp["ln1_w"].astype(np.float32)),
        "l1b": _chunk_major(inp["ln1_b"].astype(np.float32)),
        "l2w": _chunk_major(inp["ln2_w"].astype(np.float32)),
        "l2b": _chunk_major(inp["ln2_b"].astype(np.float32)),
    }
    xdt = F8 if FP8_QKV else BF16
    in_maps = []
    for c in range(N_CORES):
        b, hf = c // 2, c % 2
        xT = np.ascontiguousarray(x[b].T)
        m = dict(shared)
        m["xT"] = xT.astype(xdt)
        m["xq"] = np.ascontiguousarray(
            xT[:, hf * SQ : (hf + 1) * SQ]
        ).astype(BF16)
        in_maps.append(m)
    return in_maps


def get_program():
    if "nc" not in _CACHE:
        _CACHE["nc"] = _build()
    return _CACHE["nc"]


def kernel(**inputs):
    from concourse.bass_utils import run_bass_kernel_spmd

    nc = get_program()
    in_maps = prepare_in_maps(inputs)
    res = run_bass_kernel_spmd(nc, in_maps, core_ids=list(range(N_CORES)))
    out = np.empty((B, S, H), np.float32)
    for c in range(N_CORES):
        b, hf = c // 2, c % 2
        out[b, hf * SQ : (hf + 1) * SQ] = res.results[c]["y"]
    return out


# revision 44
# speedup vs baseline: 1.0463x; 1.0463x over previous
"""BertBlock kernel for 8 Trainium2 NeuronCores.

Sharding: pure data-parallel over (batch, half-sequence) tokens: core c
handles batch element c//2, query-token half c%2 (1024 tokens). Each core
recomputes K/V for the full 2048-token sequence of its batch element (the
duplicated K/V projection work is far cheaper than any 2-rank collective),
so no collectives are needed at all.

Device layout is feature-major ([feature, token]) end to end. The large
projections (QKV / O / MLP) run in fp8e4m3 with DoubleRow perf mode (two
128-deep contraction tiles per PE pass); weights are pre-scaled by 64 on
the host so they sit in fp8's normal range, and the 1/64 descale is folded
into the PSUM-drain ops. Scores and (optionally) AV stay bf16. Softmax
denominators come from an extra ones-column in the attention-V stationary
operand; the per-head divide uses a fast-approx DVE reciprocal and a
GPSIMD partition-broadcast so the Act engine runs exp back-to-back and
the PE never blocks on normalization.
"""

import numpy as np
import ml_dtypes

P = 128
B = 4
S = 2048          # sequence length (keys)
SQ = 1024         # query tokens per core
H = 768
HC = H // P       # 6 feature chunks
NH = 12
DH = 64
FF = 3072
FC = FF // P      # 24
TS = S // P       # 16 key-token chunks
TQ = SQ // P      # 8 query-token chunks
N_CORES = 8
EPS = 1e-5
BF16 = ml_dtypes.bfloat16
F8 = ml_dtypes.float8_e4m3
WSCALE = 64.0     # host-side weight pre-scale for fp8

# fp8 toggles per matmul group
FP8_QKV = True
FP8_O = True
FP8_MLP1 = True
FP8_MLP2 = False   # W2/h quantization is the largest rel-err contributor
FP8_AV = True

DEBUG_DUMPS = False  # adds intermediate-tensor outputs for debugging

_CACHE = {}


def _emit(nc, tc, t, mybir, make_identity):
    """Emit the per-core program. `t` maps tensor name -> DRAM AP."""
    from contextlib import ExitStack

    f32 = mybir.dt.float32
    f32r = mybir.dt.float32r
    bf16 = mybir.dt.bfloat16
    fp8 = mybir.dt.float8e4
    AF = mybir.ActivationFunctionType
    OP = mybir.AluOpType
    DR = mybir.MatmulPerfMode.DoubleRow

    def mm(ps, lhsT, rhs, start, stop, perf_mode=None):
        nc.tensor.matmul(ps, lhsT=lhsT, rhs=rhs, start=start, stop=stop,
                         perf_mode=perf_mode)

    with ExitStack() as ctx:
        aux = ctx.enter_context(tc.tile_pool(name="aux", bufs=1))

        def aux_load(name, shape, dtype=f32, eng=None):
            tl = aux.tile(shape, dtype, tag=name)
            (eng or nc.gpsimd).dma_start(tl[:], t[name])
            return tl

        bq_s = aux_load("bq2", [P, HC])
        bk_s = aux_load("bk2", [P, HC])
        bo_s = aux_load("bo2", [P, HC])
        b2_s = aux_load("b22", [P, HC])
        l1w_s = aux_load("l1w", [P, HC])
        l1b_s = aux_load("l1b", [P, HC])
        l2w_s = aux_load("l2w", [P, HC])
        l2b_s = aux_load("l2b", [P, HC])
        b1_s = aux_load("b12", [P, FC])
        bvb_s = aux.tile([P, H], bf16)
        nc.gpsimd.dma_start(bvb_s[:], t["bv"].partition_broadcast(P))
        ones_f = aux.tile([P, 1], f32)
        nc.vector.memset(ones_f[:], 1.0)
        ones_s = aux.tile([P, 1], f32r)
        nc.vector.tensor_copy(ones_s[:], ones_f[:])
        ones_b = aux.tile([P, 1], bf16)
        nc.vector.memset(ones_b[:], 1.0)
        ident_b = aux.tile([P, P], bf16)
        make_identity(nc, ident_b[:])

        qkv_dt = fp8 if FP8_QKV else bf16
        o_dt = fp8 if FP8_O else bf16
        m1_dt = fp8 if FP8_MLP1 else bf16
        m2_dt = fp8 if FP8_MLP2 else bf16
        av_dt = fp8 if FP8_AV else bf16
        qkv_sc = 1.0 / WSCALE if FP8_QKV else 1.0
        o_sc = 1.0 / WSCALE if FP8_O else 1.0
        m1_sc = 1.0 / WSCALE if FP8_MLP1 else 1.0
        m2_sc = 1.0 / WSCALE if FP8_MLP2 else 1.0

        # x1 (LN1 output) outlives the attention scopes below. bf16 is
        # plenty for the MLP residual add.
        keep = ctx.enter_context(tc.tile_pool(name="keep", bufs=1))
        x1_s = keep.tile([P, HC, SQ], bf16)
        x18_s = keep.tile([P, HC, SQ], m1_dt)
        # Wo and W1 are fully preloaded during attention (the DMA queues
        # are idle there) so the O-projection and MLP1 never stall on
        # weight streaming.
        w1p = ctx.enter_context(tc.tile_pool(name="w1_pre", bufs=1))
        w1all = w1p.tile([P, HC, FF], m1_dt)

        def proj_accum(pp, w_t, rhs_tile, rhs_lo, ps, fp8_on, n_slices,
                       tag=None):
            """Accumulate a full-contraction projection into psum `ps`.

            w_t: [P, HC, M] stationary; rhs_tile[:, kc, rhs_lo + n*512 ...]
            moving. n_slices: list of (off, width) output slices.
            """
            if fp8_on:
                for i, (off, wd) in enumerate(n_slices):
                    for kc2 in range(HC // 2):
                        mm(
                            ps[:, off : off + wd],
                            w_t[:, 2 * kc2 : 2 * kc2 + 2, :],
                            rhs_tile[
                                :, 2 * kc2 : 2 * kc2 + 2,
                                rhs_lo + off : rhs_lo + off + wd,
                            ],
                            kc2 == 0,
                            kc2 == HC // 2 - 1,
                            perf_mode=DR,
                        )
            else:
                for i, (off, wd) in enumerate(n_slices):
                    for kc in range(HC):
                        mm(
                            ps[:, off : off + wd],
                            w_t[:, kc, :],
                            rhs_tile[
                                :, kc, rhs_lo + off : rhs_lo + off + wd
                            ],
                            kc == 0,
                            kc == HC - 1,
                        )

        with tc.tile_pool(name="resid", bufs=1) as resid:
            # bf16 residual copy of this core's query tokens
            xq_s = resid.tile([P, HC, SQ], bf16)
            woall = resid.tile([P, HC, H], o_dt)
            qs = (nc.sync, nc.scalar, nc.gpsimd)
            with tc.tile_pool(name="attn_out", bufs=1) as aop:
                attnT_s = aop.tile([P, HC, SQ], o_dt)

                with tc.tile_pool(name="qkv_keep", bufs=1) as p2:
                    # qTz[p, h, q]: head h's 64 q-rows live at partitions
                    # (h%2)*64..+64 of plane h; the other 64 partitions stay
                    # zero so scores can contract over all 128 partitions.
                    qTz_s = p2.tile([P, NH, SQ], bf16)
                    nc.gpsimd.memset(qTz_s[:], 0.0)
                    kT_s = p2.tile([P, HC, S], bf16)
                    # v_s[p, kt, h*VS .. h*VS+64] = V rows for head h,
                    # col h*VS+64 = ones (softmax denominator); zero pad up
                    # to VS and at the tail lets every head take a full
                    # 128-col stationary slice v_s[:, kt, h*VS : h*VS+128].
                    # VS=128 in fp8 mode: dual-fp8 ldweights requires the
                    # kt plane stride (and safest, the per-head offsets) to
                    # be multiples of 128, so each head gets a private
                    # 128-col window.
                    VS = 128 if FP8_AV else 65
                    vcols = (NH - 1) * VS + P
                    v_s = p2.tile([P, TS, vcols], av_dt)
                    v_view = v_s[:, :, 0 : NH * VS].rearrange(
                        "p t (h d) -> p t h d", h=NH
                    )
                    nc.vector.memset(v_view[:, :, :, DH : DH + 1], 1.0)
                    if VS > DH + 1:
                        nc.gpsimd.memset(v_view[:, :, :, DH + 1 :], 0.0)
                    if vcols > NH * VS:
                        nc.gpsimd.memset(v_s[:, :, NH * VS :], 0.0)

                    # ------- QKV + attention (interleaved emission) -------
                    # Q rides a tiny fp8 query slice so the PE starts within
                    # ~15us; V consumes xT chunk pairs as they land; each K
                    # chunk is emitted just before the head pair that needs
                    # it so the exp stream (the attention bottleneck) starts
                    # as early as possible.
                    with tc.tile_pool(name="qkvph", bufs=1) as ph, tc.tile_pool(
                        name="wstream", bufs=3
                    ) as ws, tc.tile_pool(name="attn_sb", bufs=1) as ab, tc.tile_pool(
                        name="probs", bufs=3
                    ) as prp, tc.tile_pool(
                        name="work_ps", bufs=2, space="PSUM"
                    ) as pp, tc.tile_pool(
                        name="av_ps", bufs=2, space="PSUM"
                    ) as ppa:
                        xq8_s = ph.tile([P, HC, SQ], qkv_dt)
                        for j in range(HC):
                            qs[j % 3].dma_start(
                                xq8_s[:, j, :],
                                t["xq8"].rearrange("(c p) s -> p c s", p=P)[
                                    :, j, :
                                ],
                            )
                        xT_s = ph.tile([P, HC, S], qkv_dt)
                        wv_t = ws.tile([P, HC, H], qkv_dt, tag="wv", bufs=1)
                        wv_r = t["Wv"].rearrange("(c p) m -> p c m", p=P)
                        nc.sync.dma_start(wv_t[:, 0:3, :], wv_r[:, 0:3, :])
                        nc.scalar.dma_start(wv_t[:, 3:6, :], wv_r[:, 3:6, :])
                        for j in range(HC):
                            qs[(j + 2) % 3].dma_start(
                                xT_s[:, j, :],
                                t["xT"].rearrange("(c p) s -> p c s", p=P)[
                                    :, j, :
                                ],
                            )

                        # Q projection
                        for j in range(HC):
                            w_t = ws.tile([P, HC, P], qkv_dt, tag="wq")
                            qs[j % 3].dma_start(
                                w_t[:],
                                t["Wq"][:, j * P : (j + 1) * P].rearrange(
                                    "(c p) m -> p c m", p=P
                                ),
                            )
                            ps = pp.tile([P, SQ], f32, tag="wps")
                            proj_accum(
                                pp, w_t, xq8_s, 0, ps, FP8_QKV,
                                [(0, 512), (512, 512)],
                            )
                            nc.vector.tensor_scalar(
                                qTz_s[0:DH, 2 * j, :], ps[0:DH, :],
                                qkv_sc, bq_s[0:DH, j : j + 1],
                                OP.mult, OP.add,
                            )
                            nc.vector.tensor_scalar(
                                qTz_s[DH:P, 2 * j + 1, :], ps[DH:P, :],
                                qkv_sc, bq_s[DH:P, j : j + 1],
                                OP.mult, OP.add,
                            )

                        # V projection (token-major with per-head ones col)
                        for tt in range(TS):
                            ps = pp.tile([P, SQ], f32, tag="wps")
                            if FP8_QKV:
                                for kc2 in range(HC // 2):
                                    for off, wd in ((0, 512), (512, 256)):
                                        mm(
                                            ps[:, off : off + wd],
                                            xT_s[
                                                :, 2 * kc2 : 2 * kc2 + 2,
                                                tt * P : (tt + 1) * P,
                                            ],
                                            wv_t[
                                                :, 2 * kc2 : 2 * kc2 + 2,
                                                off : off + wd,
                                            ],
                                            kc2 == 0,
                                            kc2 == HC // 2 - 1,
                                            perf_mode=DR,
                                        )
                            else:
                                for kc in range(HC):
                                    for off, wd in ((0, 512), (512, 256)):
                                        mm(
                                            ps[:, off : off + wd],
                                            xT_s[:, kc, tt * P : (tt + 1) * P],
                                            wv_t[:, kc, off : off + wd],
                                            kc == 0,
                                            kc == HC - 1,
                                        )
                            nc.vector.scalar_tensor_tensor(
                                out=v_view[:, tt, :, 0:DH],
                                in0=ps[:, 0:H].rearrange("p (h d) -> p h d", h=NH),
                                scalar=qkv_sc,
                                in1=bvb_s[:].rearrange("p (h d) -> p h d", h=NH),
                                op0=OP.mult,
                                op1=OP.add,
                            )

                        # residual x and the O/MLP1 weights transfer during
                        # attention on the now-idle queues
                        for j in range(HC):
                            qs[j % 3].dma_start(
                                xq_s[:, j, :],
                                t["xq"].rearrange("(c p) s -> p c s", p=P)[
                                    :, j, :
                                ],
                            )
                        nc.gpsimd.dma_start(
                            woall[:], t["Wo"].rearrange("(c p) m -> p c m", p=P)
                        )
                        w1_r = t["W1"].rearrange("(c p) n -> p c n", p=P)
                        nc.sync.dma_start(
                            w1all[:, :, 0 : FF // 2], w1_r[:, :, 0 : FF // 2]
                        )
                        nc.scalar.dma_start(
                            w1all[:, :, FF // 2 :], w1_r[:, :, FF // 2 :]
                        )

                        avs = {}
                        spills = {}

                        def spill_head(h):
                            # Raw accumulator (attn rows) plus the sums row
                            # straight to SBUF on the DVE so the psum slot
                            # frees fast and Act stays exp-only. The sums row
                            # lands on partition 0: the fast-reciprocal
                            # custom DVE op cannot shift partitions.
                            av = avs.pop(h)
                            raw = ab.tile([DH, SQ], f32, tag="raw", bufs=3)
                            nc.vector.tensor_copy(raw[:], av[0:DH, :])
                            sums = ab.tile([1, SQ], f32, tag="sums", bufs=3)
                            nc.vector.tensor_copy(sums[:], av[DH : DH + 1, :])
                            spills[h] = (raw, sums)

                        def normalize_head(h):
                            """Divide head h's attention rows by the softmax
                            sums and place them into attnT.  Emitted one head
                            behind the matmul stream; touches no PSUM so the
                            PE never waits on it."""
                            hc = h // 2
                            raw, sums = spills.pop(h)
                            rec = ab.tile([1, SQ], f32, tag="rec", bufs=2)
                            with nc.allow_low_precision(
                                reason="softmax denominators are O(100) and "
                                "smooth; 18-bit reciprocal is plenty"
                            ):
                                nc.vector.reciprocal_approx_fast(
                                    out=rec[:], in_=sums[:]
                                )
                            bc = ab.tile([DH, SQ], f32, tag="bc", bufs=2)
                            nc.gpsimd.partition_broadcast(bc[:], rec[:])
                            if h % 2 == 0:
                                nc.vector.tensor_tensor(
                                    attnT_s[0:DH, hc, :], raw[0:DH, :],
                                    bc[:], OP.mult,
                                )
                            else:
                                tmp = ab.tile([DH, SQ], o_dt, tag="tmp", bufs=2)
                                nc.vector.tensor_tensor(
                                    tmp[:], raw[0:DH, :], bc[:], OP.mult
                                )
                                nc.sync.dma_start(
                                    attnT_s[DH:P, hc, :], tmp[:]
                                )

                        def emit_av(h, av, ktp, pr):
                            # pr: [P, 2, SQ] exp tile pair (kt = 2*ktp, +1)
                            if FP8_AV:
                                for n in range(2):
                                    mm(
                                        av[:, n * 512 : (n + 1) * 512],
                                        v_s[
                                            :,
                                            2 * ktp : 2 * ktp + 2,
                                            h * VS : h * VS + P,
                                        ],
                                        pr[:, :, n * 512 : (n + 1) * 512],
                                        ktp == 0,
                                        ktp == TS // 2 - 1,
                                        perf_mode=DR,
                                    )
                            else:
                                for i in range(2):
                                    for n in range(2):
                                        mm(
                                            av[:, n * 512 : (n + 1) * 512],
                                            v_s[
                                                :,
                                                2 * ktp + i,
                                                h * VS : h * VS + P,
                                            ],
                                            pr[:, i, n * 512 : (n + 1) * 512],
                                            ktp == 0 and i == 0,
                                            ktp == TS // 2 - 1 and i == 1,
                                        )

                        def emit_k(j):
                            wk_t = ws.tile([P, HC, P], qkv_dt, tag="wk")
                            qs[(j + 1) % 3].dma_start(
                                wk_t[:],
                                t["Wk"][:, j * P : (j + 1) * P].rearrange(
                                    "(c p) m -> p c m", p=P
                                ),
                            )
                            for hf in range(2):
                                ps = pp.tile([P, SQ], f32, tag="wps")
                                proj_accum(
                                    pp, wk_t, xT_s, hf * SQ, ps, FP8_QKV,
                                    [(0, 512), (512, 512)],
                                )
                                nc.vector.tensor_scalar(
                                    kT_s[:, j, hf * SQ : (hf + 1) * SQ],
                                    ps[:], qkv_sc, bk_s[:, j : j + 1],
                                    OP.mult, OP.add,
                                )

                        for h in range(NH):
                            hc = h // 2
                            if h % 2 == 0:
                                emit_k(hc)
                            av = ppa.tile([P, SQ], f32, tag="av")
                            avs[h] = av
                            pending = []
                            for ktp in range(TS // 2):
                                pr = prp.tile([P, 2, SQ], av_dt, tag="pr")
                                for i in range(2):
                                    kt = 2 * ktp + i
                                    sc = pp.tile([P, SQ], f32, tag="wps")
                                    lhsT_k = kT_s[
                                        :, hc, kt * P : (kt + 1) * P
                                    ]
                                    for n in range(2):
                                        mm(
                                            sc[:, n * 512 : (n + 1) * 512],
                                            lhsT_k,
                                            qTz_s[
                                                :, h, n * 512 : (n + 1) * 512
                                            ],
                                            True,
                                            True,
                                        )
                                    nc.scalar.activation(
                                        pr[:, i, :], sc[:], AF.Exp,
                                        scale=0.125,
                                    )
                                pending.append((ktp, pr))
                                if len(pending) > 1:
                                    emit_av(h, av, *pending.pop(0))
                            for p_ in pending:
                                emit_av(h, av, *p_)
                            spill_head(h)
                            if h > 0:
                                normalize_head(h - 1)
                        normalize_head(NH - 1)

                if DEBUG_DUMPS:
                    nc.sync.dma_start(t["dattn"], attnT_s[:])

                # ---------------- O-projection + residual + LN1 ----------------
                with tc.tile_pool(name="oproj", bufs=1) as op_, tc.tile_pool(
                    name="o_ps", bufs=2, space="PSUM"
                ) as ppo, tc.tile_pool(
                    name="st_ps", bufs=1, space="PSUM"
                ) as ppst:
                    r1_s = op_.tile([P, HC, SQ], f32r)
                    sum_ps = ppst.tile([1, SQ], f32, tag="lnsum", bufs=1)
                    sq_ps = ppst.tile([1, SQ], f32, tag="lnsq", bufs=1)
                    for j in range(HC):
                        ps = ppo.tile([P, SQ], f32, tag="ops")
                        proj_accum(
                            ppo, woall[:, :, j * P : (j + 1) * P],
                            attnT_s, 0, ps, FP8_O,
                            [(0, 512), (512, 512)],
                        )
                        to = op_.tile([P, SQ], f32, tag="to", bufs=2)
                        nc.scalar.activation(
                            to[:], ps[:], AF.Identity,
                            bias=bo_s[:, j : j + 1], scale=o_sc,
                        )
                        nc.vector.tensor_tensor(
                            r1_s[:, j, :], to[:], xq_s[:, j, :], OP.add
                        )
                        sq_t = op_.tile([P, SQ], f32r, tag="sqt", bufs=2)
                        nc.vector.tensor_tensor(
                            sq_t[:], r1_s[:, j, :], r1_s[:, j, :], OP.mult
                        )
                        for n in range(2):
                            mm(
                                sum_ps[:, n * 512 : (n + 1) * 512],
                                ones_s[:],
                                r1_s[:, j, n * 512 : (n + 1) * 512],
                                j == 0, j == HC - 1,
                            )
                            mm(
                                sq_ps[:, n * 512 : (n + 1) * 512],
                                ones_s[:],
                                sq_t[:, n * 512 : (n + 1) * 512],
                                j == 0, j == HC - 1,
                            )
                    if DEBUG_DUMPS:
                        nc.sync.dma_start(
                            t["dr1"], r1_s[:].bitcast(f32)
                        )
                    _ln_normalize(
                        nc, tc, mybir, op_, sum_ps, sq_ps, r1_s,
                        [x1_s, x18_s], l1w_s, l1b_s,
                    )
                    if DEBUG_DUMPS:
                        nc.sync.dma_start(t["dx1"], x1_s[:])

        # ---------------- MLP + LN2 + output ----------------
        with tc.tile_pool(name="mlp", bufs=1) as mp, tc.tile_pool(
            name="st2_ps", bufs=1, space="PSUM"
        ) as ppst2:
            hT_s = mp.tile([P, FC, SQ], m2_dt)
            r2_s = mp.tile([P, HC, SQ], bf16)
            # W2 in two halves so MLP2's first chunks don't wait for the
            # whole tensor; both transfer during the LN1 tail + MLP1.
            w2_s = mp.tile([P, FC, H], m2_dt)
            w2_r = t["W2"].rearrange("(c p) m -> p c m", p=P)
            nc.sync.dma_start(w2_s[:, 0 : FC // 2, :], w2_r[:, 0 : FC // 2, :])
            nc.scalar.dma_start(w2_s[:, FC // 2 :, :], w2_r[:, FC // 2 :, :])
            sum2_ps = ppst2.tile([1, SQ], f32, tag="ln2sum", bufs=1)
            sq2_ps = ppst2.tile([1, SQ], f32, tag="ln2sq", bufs=1)
            with tc.tile_pool(name="m_ps", bufs=2, space="PSUM") as ppm:
                for m in range(FC):
                    ps = ppm.tile([P, SQ], f32, tag="mps")
                    proj_accum(
                        ppm, w1all[:, :, m * P : (m + 1) * P], x18_s, 0,
                        ps, FP8_MLP1, [(0, 512), (512, 512)],
                    )
                    nc.scalar.activation(
                        hT_s[:, m, :], ps[:], AF.Gelu,
                        bias=b1_s[:, m : m + 1], scale=m1_sc,
                    )

                for j in range(HC):
                    ps = ppm.tile([P, SQ], f32, tag="mps")
                    if FP8_MLP2:
                        for n in range(2):
                            for kc2 in range(FC // 2):
                                mm(
                                    ps[:, n * 512 : (n + 1) * 512],
                                    w2_s[
                                        :, 2 * kc2 : 2 * kc2 + 2,
                                        j * P : (j + 1) * P,
                                    ],
                                    hT_s[
                                        :, 2 * kc2 : 2 * kc2 + 2,
                                        n * 512 : (n + 1) * 512,
                                    ],
                                    kc2 == 0,
                                    kc2 == FC // 2 - 1,
                                    perf_mode=DR,
                                )
                    else:
                        for n in range(2):
                            for kc in range(FC):
                                mm(
                                    ps[:, n * 512 : (n + 1) * 512],
                                    w2_s[:, kc, j * P : (j + 1) * P],
                                    hT_s[:, kc, n * 512 : (n + 1) * 512],
                                    kc == 0,
                                    kc == FC - 1,
                                )
                    to = mp.tile([P, SQ], f32, tag="to2", bufs=1)
                    nc.scalar.activation(
                        to[:], ps[:], AF.Identity,
                        bias=b2_s[:, j : j + 1], scale=m2_sc,
                    )
                    nc.vector.tensor_tensor(
                        r2_s[:, j, :], to[:], x1_s[:, j, :], OP.add
                    )
                    sq_t = mp.tile([P, SQ], bf16, tag="sqt2", bufs=2)
                    nc.vector.tensor_tensor(
                        sq_t[:], r2_s[:, j, :], r2_s[:, j, :], OP.mult
                    )
                    for n in range(2):
                        mm(
                            sum2_ps[:, n * 512 : (n + 1) * 512],
                            ones_b[:],
                            r2_s[:, j, n * 512 : (n + 1) * 512],
                            j == 0, j == HC - 1,
                        )
                        mm(
                            sq2_ps[:, n * 512 : (n + 1) * 512],
                            ones_b[:],
                            sq_t[:, n * 512 : (n + 1) * 512],
                            j == 0, j == HC - 1,
                        )
                if DEBUG_DUMPS:
                    nc.sync.dma_start(t["dh"], hT_s[:])
                    nc.sync.dma_start(t["dr2"], r2_s[:])

            # LN2 normalizes r2 in place; each chunk is transposed to
            # token-major as soon as it is normalized, and each token tile
            # is stored as soon as its last chunk lands.
            with tc.tile_pool(name="outp", bufs=1) as outp, tc.tile_pool(
                name="tr_ps", bufs=4, space="PSUM"
            ) as ppt:
                out_all = outp.tile([P, TQ, H], bf16)

                def ln2_chunk(j):
                    for tt in range(TQ):
                        tps = ppt.tile([P, P], bf16, tag="tr")
                        nc.tensor.transpose(
                            tps[:],
                            r2_s[:, j, tt * P : (tt + 1) * P],
                            ident_b[:],
                        )
                        if tt % 2 == 0:
                            nc.scalar.activation(
                                out_all[:, tt, j * P : (j + 1) * P], tps[:],
                                AF.Identity,
                            )
                        else:
                            nc.vector.tensor_copy(
                                out_all[:, tt, j * P : (j + 1) * P], tps[:]
                            )

                _ln_normalize(
                    nc, tc, mybir, mp, sum2_ps, sq2_ps, r2_s, [r2_s],
                    l2w_s, l2b_s, chunk_cb=ln2_chunk,
                )
                for tt in range(TQ):
                    qs[tt % 3].dma_start(
                        t["y"][tt * P : (tt + 1) * P, :], out_all[:, tt, :]
                    )


def _ln_normalize(nc, tc, mybir, pool, sum_ps, sq_ps, src_s, dsts, w_s, b_s,
                  chunk_cb=None):
    """Feature-major LayerNorm given accumulated sum / sum-of-squares rows.

    src_s: [P, HC, SQ]. Stats ops run on [1, SQ] rows; mean/rstd broadcast
    across partitions on the GPSIMD engine; the two tensor-tensor steps run
    on the DVE; the final scale-bias (with dtype cast) runs on the Act
    engine once per entry in `dsts` (each a [P, HC, SQ] AP). chunk_cb(j)
    is called after chunk j's outputs are written (for pipelining)."""
    f32 = mybir.dt.float32
    AF = mybir.ActivationFunctionType
    OP = mybir.AluOpType

    f32r = mybir.dt.float32r

    def fm_slice(j):
        sl = src_s[:, j, :]
        return sl.bitcast(f32) if src_s.dtype == f32r else sl

    # Stats chain spread across engines: mean on Act, its broadcast on
    # GPSIMD immediately after, the variance/reciprocal smalls on DVE,
    # rstd sqrt back on Act, its broadcast on GPSIMD. The (src - mean)
    # step for half the chunks runs on GPSIMD between the two
    # broadcasts, in parallel with the DVE smalls.
    mean = pool.tile([1, SQ], f32, tag="lnmean", bufs=1)
    nc.scalar.activation(mean[:], sum_ps[:], AF.Identity, scale=1.0 / H)
    mb = pool.tile([P, SQ], f32, tag="lnmb", bufs=1)
    nc.gpsimd.partition_broadcast(mb[:], mean[:])
    m2 = pool.tile([1, SQ], f32, tag="lnsm", bufs=2)
    nc.vector.tensor_tensor(m2[:], mean[:], mean[:], OP.mult)
    vpe = pool.tile([1, SQ], f32, tag="lnsm", bufs=2)
    nc.vector.scalar_tensor_tensor(
        out=vpe[:], in0=sq_ps[:], scalar=1.0 / H, in1=m2[:], op0=OP.mult,
        op1=OP.subtract,
    )
    nc.vector.tensor_scalar_add(vpe[:], vpe[:], EPS)
    rvar = pool.tile([1, SQ], f32, tag="lnsm", bufs=2)
    with nc.allow_low_precision(reason="18-bit 1/(var+eps) is benign"):
        nc.vector.reciprocal_approx_fast(out=rvar[:], in_=vpe[:])
    rstd = pool.tile([1, SQ], f32, tag="lnrstd", bufs=1)
    nc.scalar.activation(rstd[:], rvar[:], AF.Sqrt)
    rb = pool.tile([P, SQ], f32, tag="lnrb", bufs=1)
    nc.gpsimd.partition_broadcast(rb[:], rstd[:])
    for j in range(HC):
        t1 = pool.tile([P, SQ], f32, tag="lnt1v", bufs=2)
        nc.vector.tensor_tensor(t1[:], fm_slice(j), mb[:], OP.subtract)
        t2 = pool.tile([P, SQ], f32, tag="lnt2", bufs=2)
        nc.vector.tensor_tensor(t2[:], t1[:], rb[:], OP.mult)
        for dst in dsts:
            nc.scalar.activation(
                dst[:, j, :], t2[:], AF.Identity,
                bias=b_s[:, j : j + 1], scale=w_s[:, j : j + 1],
            )
        if chunk_cb is not None:
            chunk_cb(j)


def _build():
    import concourse.bacc as bacc
    import concourse.tile as tile
    import concourse.mybir as mybir
    from concourse.masks import make_identity

    f32 = mybir.dt.float32
    bf16 = mybir.dt.bfloat16
    fp8 = mybir.dt.float8e4

    qkv_dt = fp8 if FP8_QKV else bf16
    o_dt = fp8 if FP8_O else bf16
    m1_dt = fp8 if FP8_MLP1 else bf16
    m2_dt = fp8 if FP8_MLP2 else bf16

    nc = bacc.Bacc(
        "TRN2", target_bir_lowering=False, debug=False, num_devices=N_CORES
    )
    specs = [
        ("xT", [H, S], qkv_dt, "ExternalInput"),
        ("xq", [H, SQ], bf16, "ExternalInput"),
        ("xq8", [H, SQ], qkv_dt, "ExternalInput"),
        ("Wq", [H, H], qkv_dt, "ExternalInput"),
        ("Wk", [H, H], qkv_dt, "ExternalInput"),
        ("Wv", [H, H], qkv_dt, "ExternalInput"),
        ("Wo", [H, H], o_dt, "ExternalInput"),
        ("W1", [H, FF], m1_dt, "ExternalInput"),
        ("W2", [FF, H], m2_dt, "ExternalInput"),
        ("bq2", [P, HC], f32, "ExternalInput"),
        ("bk2", [P, HC], f32, "ExternalInput"),
        ("bv", [H], f32, "ExternalInput"),
        ("bo2", [P, HC], f32, "ExternalInput"),
        ("b12", [P, FC], f32, "ExternalInput"),
        ("b22", [P, HC], f32, "ExternalInput"),
        ("l1w", [P, HC], f32, "ExternalInput"),
        ("l1b", [P, HC], f32, "ExternalInput"),
        ("l2w", [P, HC], f32, "ExternalInput"),
        ("l2b", [P, HC], f32, "ExternalInput"),
        ("y", [SQ, H], bf16, "ExternalOutput"),
    ]
    if DEBUG_DUMPS:
        av_dt = fp8 if FP8_AV else bf16
        vs = 128 if FP8_AV else 65
        vcols = (12 - 1) * vs + P
        specs += [
            ("dq", [P, NH, SQ], bf16, "ExternalOutput"),
            ("dk", [P, HC, S], bf16, "ExternalOutput"),
            ("dv", [P, 16, vcols], av_dt, "ExternalOutput"),
            ("dattn", [P, HC, SQ], o_dt, "ExternalOutput"),
            ("dr1", [P, HC, SQ], f32, "ExternalOutput"),
            ("dx1", [P, HC, SQ], bf16, "ExternalOutput"),
            ("dh", [P, FC, SQ], m2_dt, "ExternalOutput"),
            ("dr2", [P, HC, SQ], bf16, "ExternalOutput"),
        ]
    t = {
        name: nc.dram_tensor(name, shape, dt, kind=kind).ap()
        for name, shape, dt, kind in specs
    }
    with tile.TileContext(nc) as tc:
        _emit(nc, tc, t, mybir, make_identity)
    nc.compile()
    return nc


def _chunk_major(v):
    """[C*P] -> [P, C] with entry [p, c] = v[c*P + p]."""
    return np.ascontiguousarray(v.reshape(-1, P).T)


def prepare_in_maps(inputs):
    inp = {k: np.asarray(v) for k, v in inputs.items()}
    x = inp["x"].astype(np.float32)

    def wcast(w, on):
        w = w.astype(np.float32)
        if on:
            return (w * WSCALE).astype(F8)
        return w.astype(BF16)

    shared = {
        "Wq": wcast(inp["Wq"], FP8_QKV),
        "Wk": wcast(inp["Wk"], FP8_QKV),
        "Wv": wcast(inp["Wv"], FP8_QKV),
        "Wo": wcast(inp["Wo"], FP8_O),
        "W1": wcast(inp["W1"], FP8_MLP1),
        "W2": wcast(inp["W2"], FP8_MLP2),
        "bq2": _chunk_major(inp["bq"].astype(np.float32)),
        "bk2": _chunk_major(inp["bk"].astype(np.float32)),
        "bv": inp["bv"].astype(np.float32),
        "bo2": _chunk_major(inp["bo"].astype(np.float32)),
        "b12": _chunk_major(inp["b1"].astype(np.float32)),
        "b22": _chunk_major(inp["b2"].astype(np.float32)),
        "l1w": _chunk_major(inp["ln1_w"].astype(np.float32)),
        "l1b": _chunk_major(inp["ln1_b"].astype(np.float32)),
        "l2w": _chunk_major(inp["ln2_w"].astype(np.float32)),
        "l2b": _chunk_major(inp["ln2_b"].astype(np.float32)),
    }
    xdt = F8 if FP8_QKV else BF16
    in_maps = []
    for c in range(N_CORES):
        b, hf = c // 2, c % 2
        xT = np.ascontiguousarray(x[b].T)
        m = dict(shared)
        m["xT"] = xT.astype(xdt)
        xqs = np.ascontiguousarray(xT[:, hf * SQ : (hf + 1) * SQ])
        m["xq"] = xqs.astype(BF16)
        m["xq8"] = xqs.astype(xdt)
        in_maps.append(m)
    return in_maps


def get_program():
    if "nc" not in _CACHE:
        _CACHE["nc"] = _build()
    return _CACHE["nc"]


def kernel(**inputs):
    from concourse.bass_utils import run_bass_kernel_spmd

    nc = get_program()
    in_maps = prepare_in_maps(inputs)
    res = run_bass_kernel_spmd(nc, in_maps, core_ids=list(range(N_CORES)))
    out = np.empty((B, S, H), np.float32)
    for c in range(N_CORES):
        b, hf = c // 2, c % 2
        out[b, hf * SQ : (hf + 1) * SQ] = res.results[c]["y"]
    return out


# revision 47
# speedup vs baseline: 1.0615x; 1.0145x over previous
"""BertBlock kernel for 8 Trainium2 NeuronCores.

Sharding: pure data-parallel over (batch, half-sequence) tokens: core c
handles batch element c//2, query-token half c%2 (1024 tokens). Each core
recomputes K/V for the full 2048-token sequence of its batch element (the
duplicated K/V projection work is far cheaper than any 2-rank collective),
so no collectives are needed at all.

Device layout is feature-major ([feature, token]) end to end. The large
projections (QKV / O / MLP) run in fp8e4m3 with DoubleRow perf mode (two
128-deep contraction tiles per PE pass); weights are pre-scaled by 64 on
the host so they sit in fp8's normal range, and the 1/64 descale is folded
into the PSUM-drain ops. Scores and (optionally) AV stay bf16. Softmax
denominators come from an extra ones-column in the attention-V stationary
operand; the per-head divide uses a fast-approx DVE reciprocal and a
GPSIMD partition-broadcast so the Act engine runs exp back-to-back and
the PE never blocks on normalization.
"""

import numpy as np
import ml_dtypes

P = 128
B = 4
S = 2048          # sequence length (keys)
SQ = 1024         # query tokens per core
H = 768
HC = H // P       # 6 feature chunks
NH = 12
DH = 64
FF = 3072
FC = FF // P      # 24
TS = S // P       # 16 key-token chunks
TQ = SQ // P      # 8 query-token chunks
N_CORES = 8
EPS = 1e-5
BF16 = ml_dtypes.bfloat16
F8 = ml_dtypes.float8_e4m3
WSCALE = 64.0     # host-side weight pre-scale for fp8

# fp8 toggles per matmul group
FP8_QKV = True
FP8_O = True
FP8_MLP1 = True
FP8_MLP2 = False   # W2/h quantization is the largest rel-err contributor
FP8_AV = True

DEBUG_DUMPS = False  # adds intermediate-tensor outputs for debugging

_CACHE = {}


def _emit(nc, tc, t, mybir, make_identity):
    """Emit the per-core program. `t` maps tensor name -> DRAM AP."""
    from contextlib import ExitStack

    f32 = mybir.dt.float32
    f32r = mybir.dt.float32r
    bf16 = mybir.dt.bfloat16
    fp8 = mybir.dt.float8e4
    AF = mybir.ActivationFunctionType
    OP = mybir.AluOpType
    DR = mybir.MatmulPerfMode.DoubleRow

    def mm(ps, lhsT, rhs, start, stop, perf_mode=None):
        nc.tensor.matmul(ps, lhsT=lhsT, rhs=rhs, start=start, stop=stop,
                         perf_mode=perf_mode)

    with ExitStack() as ctx:
        aux = ctx.enter_context(tc.tile_pool(name="aux", bufs=1))

        def aux_load(name, shape, dtype=f32, eng=None):
            tl = aux.tile(shape, dtype, tag=name)
            (eng or nc.gpsimd).dma_start(tl[:], t[name])
            return tl

        bq_s = aux_load("bq2", [P, HC])
        bk_s = aux_load("bk2", [P, HC])
        bo_s = aux_load("bo2", [P, HC])
        b2_s = aux_load("b22", [P, HC])
        l1w_s = aux_load("l1w", [P, HC])
        l1b_s = aux_load("l1b", [P, HC])
        l2w_s = aux_load("l2w", [P, HC])
        l2b_s = aux_load("l2b", [P, HC])
        b1_s = aux_load("b12", [P, FC])
        bvb_s = aux.tile([P, H], bf16)
        nc.gpsimd.dma_start(bvb_s[:], t["bv"].partition_broadcast(P))
        ones_f = aux.tile([P, 1], f32)
        nc.vector.memset(ones_f[:], 1.0)
        ones_s = aux.tile([P, 1], f32r)
        nc.vector.tensor_copy(ones_s[:], ones_f[:])
        ones_b = aux.tile([P, 1], bf16)
        nc.vector.memset(ones_b[:], 1.0)
        ident_b = aux.tile([P, P], bf16)
        make_identity(nc, ident_b[:])

        qkv_dt = fp8 if FP8_QKV else bf16
        o_dt = fp8 if FP8_O else bf16
        m1_dt = fp8 if FP8_MLP1 else bf16
        m2_dt = fp8 if FP8_MLP2 else bf16
        av_dt = fp8 if FP8_AV else bf16
        qkv_sc = 1.0 / WSCALE if FP8_QKV else 1.0
        o_sc = 1.0 / WSCALE if FP8_O else 1.0
        m1_sc = 1.0 / WSCALE if FP8_MLP1 else 1.0
        m2_sc = 1.0 / WSCALE if FP8_MLP2 else 1.0

        # x1 (LN1 output) outlives the attention scopes below. bf16 is
        # plenty for the MLP residual add.
        keep = ctx.enter_context(tc.tile_pool(name="keep", bufs=1))
        x1_s = keep.tile([P, HC, SQ], bf16)
        x18_s = keep.tile([P, HC, SQ], m1_dt)
        # Wo and W1 are fully preloaded during attention (the DMA queues
        # are idle there) so the O-projection and MLP1 never stall on
        # weight streaming.
        w1p = ctx.enter_context(tc.tile_pool(name="w1_pre", bufs=1))
        w1all = w1p.tile([P, HC, FF], m1_dt)

        def proj_accum(pp, w_t, rhs_tile, rhs_lo, ps, fp8_on, n_slices,
                       tag=None):
            """Accumulate a full-contraction projection into psum `ps`.

            w_t: [P, HC, M] stationary; rhs_tile[:, kc, rhs_lo + n*512 ...]
            moving. n_slices: list of (off, width) output slices.
            """
            if fp8_on:
                for i, (off, wd) in enumerate(n_slices):
                    for kc2 in range(HC // 2):
                        mm(
                            ps[:, off : off + wd],
                            w_t[:, 2 * kc2 : 2 * kc2 + 2, :],
                            rhs_tile[
                                :, 2 * kc2 : 2 * kc2 + 2,
                                rhs_lo + off : rhs_lo + off + wd,
                            ],
                            kc2 == 0,
                            kc2 == HC // 2 - 1,
                            perf_mode=DR,
                        )
            else:
                for i, (off, wd) in enumerate(n_slices):
                    for kc in range(HC):
                        mm(
                            ps[:, off : off + wd],
                            w_t[:, kc, :],
                            rhs_tile[
                                :, kc, rhs_lo + off : rhs_lo + off + wd
                            ],
                            kc == 0,
                            kc == HC - 1,
                        )

        with tc.tile_pool(name="resid", bufs=1) as resid:
            # bf16 residual copy of this core's query tokens
            xq_s = resid.tile([P, HC, SQ], bf16)
            woall = resid.tile([P, HC, H], o_dt)
            qs = (nc.sync, nc.scalar, nc.gpsimd)
            with tc.tile_pool(name="attn_out", bufs=1) as aop:
                attnT_s = aop.tile([P, HC, SQ], o_dt)

                with tc.tile_pool(name="qkv_keep", bufs=1) as p2:
                    # qTz[p, h, q]: head h's 64 q-rows live at partitions
                    # (h%2)*64..+64 of plane h; the other 64 partitions stay
                    # zero so scores can contract over all 128 partitions.
                    qTz_s = p2.tile([P, NH, SQ], bf16)
                    nc.gpsimd.memset(qTz_s[:], 0.0)
                    kT_s = p2.tile([P, HC, S], bf16)
                    # v_s[p, kt, h*VS .. h*VS+64] = V rows for head h,
                    # col h*VS+64 = ones (softmax denominator); zero pad up
                    # to VS and at the tail lets every head take a full
                    # 128-col stationary slice v_s[:, kt, h*VS : h*VS+128].
                    # VS=128 in fp8 mode: dual-fp8 ldweights requires the
                    # kt plane stride (and safest, the per-head offsets) to
                    # be multiples of 128, so each head gets a private
                    # 128-col window.
                    VS = 128 if FP8_AV else 65
                    vcols = (NH - 1) * VS + P
                    v_s = p2.tile([P, TS, vcols], av_dt)
                    v_view = v_s[:, :, 0 : NH * VS].rearrange(
                        "p t (h d) -> p t h d", h=NH
                    )
                    nc.vector.memset(v_view[:, :, :, DH : DH + 1], 1.0)
                    if VS > DH + 1:
                        nc.gpsimd.memset(v_view[:, :, :, DH + 1 :], 0.0)
                    if vcols > NH * VS:
                        nc.gpsimd.memset(v_s[:, :, NH * VS :], 0.0)

                    # ------- QKV + attention (interleaved emission) -------
                    # Q rides a tiny fp8 query slice so the PE starts within
                    # ~15us; V consumes xT chunk pairs as they land; each K
                    # chunk is emitted just before the head pair that needs
                    # it so the exp stream (the attention bottleneck) starts
                    # as early as possible.
                    with tc.tile_pool(name="qkvph", bufs=1) as ph, tc.tile_pool(
                        name="wstream", bufs=3
                    ) as ws, tc.tile_pool(name="attn_sb", bufs=1) as ab, tc.tile_pool(
                        name="probs", bufs=3
                    ) as prp, tc.tile_pool(
                        name="work_ps", bufs=2, space="PSUM"
                    ) as pp, tc.tile_pool(
                        name="av_ps", bufs=2, space="PSUM"
                    ) as ppa:
                        xq8_s = ph.tile([P, HC, SQ], qkv_dt)
                        for j in range(HC):
                            qs[j % 3].dma_start(
                                xq8_s[:, j, :],
                                t["xq8"].rearrange("(c p) s -> p c s", p=P)[
                                    :, j, :
                                ],
                            )
                        # weight preloads in exp-criticality order: Wq,
                        # Wk (gate the first scores/exp), then xT, Wv (gate
                        # V/AV), then the post-attention loads.
                        wqall = ws.tile([P, HC, H], qkv_dt, tag="wq", bufs=1)
                        wq_r = t["Wq"].rearrange("(c p) m -> p c m", p=P)
                        nc.sync.dma_start(wqall[:, 0:3, :], wq_r[:, 0:3, :])
                        nc.scalar.dma_start(wqall[:, 3:6, :], wq_r[:, 3:6, :])
                        wkall = ph.tile([P, HC, H], qkv_dt)
                        wk_r = t["Wk"].rearrange("(c p) m -> p c m", p=P)
                        nc.gpsimd.dma_start(wkall[:, 0:3, :], wk_r[:, 0:3, :])
                        nc.sync.dma_start(wkall[:, 3:6, :], wk_r[:, 3:6, :])
                        xT_s = ph.tile([P, HC, S], qkv_dt)
                        for j in range(HC):
                            qs[(j + 2) % 3].dma_start(
                                xT_s[:, j, :],
                                t["xT"].rearrange("(c p) s -> p c s", p=P)[
                                    :, j, :
                                ],
                            )
                        wvall = ph.tile([P, HC, H], qkv_dt)
                        wv_r = t["Wv"].rearrange("(c p) m -> p c m", p=P)
                        nc.scalar.dma_start(wvall[:, 0:3, :], wv_r[:, 0:3, :])
                        nc.gpsimd.dma_start(wvall[:, 3:6, :], wv_r[:, 3:6, :])
                        # residual x and the O/MLP1 weights transfer during
                        # attention on the then-idle queues
                        for j in range(HC):
                            qs[j % 3].dma_start(
                                xq_s[:, j, :],
                                t["xq"].rearrange("(c p) s -> p c s", p=P)[
                                    :, j, :
                                ],
                            )
                        nc.gpsimd.dma_start(
                            woall[:], t["Wo"].rearrange("(c p) m -> p c m", p=P)
                        )
                        w1_r = t["W1"].rearrange("(c p) n -> p c n", p=P)
                        nc.sync.dma_start(
                            w1all[:, :, 0 : FF // 2], w1_r[:, :, 0 : FF // 2]
                        )
                        nc.scalar.dma_start(
                            w1all[:, :, FF // 2 :], w1_r[:, :, FF // 2 :]
                        )

                        # Q projection
                        for j in range(HC):
                            ps = pp.tile([P, SQ], f32, tag="wps")
                            proj_accum(
                                pp, wqall[:, :, j * P : (j + 1) * P],
                                xq8_s, 0, ps, FP8_QKV,
                                [(0, 512), (512, 512)],
                            )
                            nc.vector.tensor_scalar(
                                qTz_s[0:DH, 2 * j, :], ps[0:DH, :],
                                qkv_sc, bq_s[0:DH, j : j + 1],
                                OP.mult, OP.add,
                            )
                            nc.vector.tensor_scalar(
                                qTz_s[DH:P, 2 * j + 1, :], ps[DH:P, :],
                                qkv_sc, bq_s[DH:P, j : j + 1],
                                OP.mult, OP.add,
                            )

                        def emit_v_tt(tt):
                            # V projection for one 128-token chunk
                            ps = pp.tile([P, SQ], f32, tag="wps")
                            if FP8_QKV:
                                for kc2 in range(HC // 2):
                                    for off, wd in ((0, 512), (512, 256)):
                                        mm(
                                            ps[:, off : off + wd],
                                            xT_s[
                                                :, 2 * kc2 : 2 * kc2 + 2,
                                                tt * P : (tt + 1) * P,
                                            ],
                                            wvall[
                                                :, 2 * kc2 : 2 * kc2 + 2,
                                                off : off + wd,
                                            ],
                                            kc2 == 0,
                                            kc2 == HC // 2 - 1,
                                            perf_mode=DR,
                                        )
                            else:
                                for kc in range(HC):
                                    for off, wd in ((0, 512), (512, 256)):
                                        mm(
                                            ps[:, off : off + wd],
                                            xT_s[:, kc, tt * P : (tt + 1) * P],
                                            wvall[:, kc, off : off + wd],
                                            kc == 0,
                                            kc == HC - 1,
                                        )
                            nc.vector.scalar_tensor_tensor(
                                out=v_view[:, tt, :, 0:DH],
                                in0=ps[:, 0:H].rearrange(
                                    "p (h d) -> p h d", h=NH
                                ),
                                scalar=qkv_sc,
                                in1=bvb_s[:].rearrange(
                                    "p (h d) -> p h d", h=NH
                                ),
                                op0=OP.mult,
                                op1=OP.add,
                            )

                        avs = {}
                        avs = {}
                        spills = {}

                        def spill_head(h):
                            # Raw accumulator (attn rows) plus the sums row
                            # straight to SBUF on the DVE so the psum slot
                            # frees fast and Act stays exp-only. The sums row
                            # lands on partition 0: the fast-reciprocal
                            # custom DVE op cannot shift partitions.
                            av = avs.pop(h)
                            raw = ab.tile([DH, SQ], f32, tag="raw", bufs=2)
                            nc.vector.tensor_copy(raw[:], av[0:DH, :])
                            sums = ab.tile([1, SQ], f32, tag="sums", bufs=2)
                            nc.vector.tensor_copy(sums[:], av[DH : DH + 1, :])
                            spills[h] = (raw, sums)

                        def normalize_head(h):
                            """Divide head h's attention rows by the softmax
                            sums and place them into attnT.  Emitted one head
                            behind the matmul stream; touches no PSUM so the
                            PE never waits on it."""
                            hc = h // 2
                            raw, sums = spills.pop(h)
                            rec = ab.tile([1, SQ], f32, tag="rec", bufs=2)
                            with nc.allow_low_precision(
                                reason="softmax denominators are O(100) and "
                                "smooth; 18-bit reciprocal is plenty"
                            ):
                                nc.vector.reciprocal_approx_fast(
                                    out=rec[:], in_=sums[:]
                                )
                            bc = ab.tile([DH, SQ], f32, tag="bc", bufs=2)
                            nc.gpsimd.partition_broadcast(bc[:], rec[:])
                            if h % 2 == 0:
                                nc.vector.tensor_tensor(
                                    attnT_s[0:DH, hc, :], raw[0:DH, :],
                                    bc[:], OP.mult,
                                )
                            else:
                                tmp = ab.tile([DH, SQ], o_dt, tag="tmp", bufs=2)
                                nc.vector.tensor_tensor(
                                    tmp[:], raw[0:DH, :], bc[:], OP.mult
                                )
                                nc.sync.dma_start(
                                    attnT_s[DH:P, hc, :], tmp[:]
                                )

                        def emit_av(h, av, ktp, pr):
                            # pr: [P, 2, SQ] exp tile pair (kt = 2*ktp, +1)
                            if FP8_AV:
                                for n in range(2):
                                    mm(
                                        av[:, n * 512 : (n + 1) * 512],
                                        v_s[
                                            :,
                                            2 * ktp : 2 * ktp + 2,
                                            h * VS : h * VS + P,
                                        ],
                                        pr[:, :, n * 512 : (n + 1) * 512],
                                        ktp == 0,
                                        ktp == TS // 2 - 1,
                                        perf_mode=DR,
                                    )
                            else:
                                for i in range(2):
                                    for n in range(2):
                                        mm(
                                            av[:, n * 512 : (n + 1) * 512],
                                            v_s[
                                                :,
                                                2 * ktp + i,
                                                h * VS : h * VS + P,
                                            ],
                                            pr[:, i, n * 512 : (n + 1) * 512],
                                            ktp == 0 and i == 0,
                                            ktp == TS // 2 - 1 and i == 1,
                                        )

                        def emit_k(j):
                            for hf in range(2):
                                ps = pp.tile([P, SQ], f32, tag="wps")
                                proj_accum(
                                    pp, wkall[:, :, j * P : (j + 1) * P],
                                    xT_s, hf * SQ, ps, FP8_QKV,
                                    [(0, 512), (512, 512)],
                                )
                                nc.vector.tensor_scalar(
                                    kT_s[:, j, hf * SQ : (hf + 1) * SQ],
                                    ps[:], qkv_sc, bk_s[:, j : j + 1],
                                    OP.mult, OP.add,
                                )

                        def se_head(h, pending):
                            """Scores + exp for one head into the pr ring."""
                            for ktp in range(TS // 2):
                                pr = prp.tile([P, 2, SQ], av_dt, tag="pr")
                                for i in range(2):
                                    kt = 2 * ktp + i
                                    sc = pp.tile([P, SQ], f32, tag="wps")
                                    lhsT_k = kT_s[
                                        :, h // 2, kt * P : (kt + 1) * P
                                    ]
                                    for n in range(2):
                                        mm(
                                            sc[:, n * 512 : (n + 1) * 512],
                                            lhsT_k,
                                            qTz_s[
                                                :, h, n * 512 : (n + 1) * 512
                                            ],
                                            True,
                                            True,
                                        )
                                    nc.scalar.activation(
                                        pr[:, i, :], sc[:], AF.Exp,
                                        scale=0.125,
                                    )
                                pending.append((ktp, pr))
                                yield

                        def drain_avs(h, av, pending, keep_depth):
                            while len(pending) > keep_depth:
                                emit_av(h, av, *pending.pop(0))

                        # Head 0 interleaves the V projection with its own
                        # scores/exp/AV stream: the PE executes in
                        # instruction order, so this puts the first exp right
                        # after K chunk 0 instead of behind the whole V
                        # projection, while AV pair k only needs the two V
                        # chunks emitted alongside it.
                        emit_k(0)
                        av0 = ppa.tile([P, SQ], f32, tag="av")
                        avs[0] = av0
                        pend0 = []
                        for ktp, _ in enumerate(se_head(0, pend0)):
                            emit_v_tt(2 * ktp)
                            emit_v_tt(2 * ktp + 1)
                            drain_avs(0, av0, pend0, 1)
                        drain_avs(0, av0, pend0, 0)
                        spill_head(0)
                        for h in range(1, NH):
                            if h % 2 == 0:
                                emit_k(h // 2)
                            av = ppa.tile([P, SQ], f32, tag="av")
                            avs[h] = av
                            pending = []
                            for _ in se_head(h, pending):
                                drain_avs(h, av, pending, 1)
                            drain_avs(h, av, pending, 0)
                            spill_head(h)
                            normalize_head(h - 1)
                        normalize_head(NH - 1)

                if DEBUG_DUMPS:
                    nc.sync.dma_start(t["dattn"], attnT_s[:])

                # ---------------- O-projection + residual + LN1 ----------------
                with tc.tile_pool(name="oproj", bufs=1) as op_, tc.tile_pool(
                    name="o_ps", bufs=2, space="PSUM"
                ) as ppo, tc.tile_pool(
                    name="st_ps", bufs=1, space="PSUM"
                ) as ppst:
                    r1_s = op_.tile([P, HC, SQ], f32r)
                    sum_ps = ppst.tile([1, SQ], f32, tag="lnsum", bufs=1)
                    sq_ps = ppst.tile([1, SQ], f32, tag="lnsq", bufs=1)
                    for j in range(HC):
                        ps = ppo.tile([P, SQ], f32, tag="ops")
                        proj_accum(
                            ppo, woall[:, :, j * P : (j + 1) * P],
                            attnT_s, 0, ps, FP8_O,
                            [(0, 512), (512, 512)],
                        )
                        to = op_.tile([P, SQ], f32, tag="to", bufs=2)
                        nc.scalar.activation(
                            to[:], ps[:], AF.Identity,
                            bias=bo_s[:, j : j + 1], scale=o_sc,
                        )
                        nc.vector.tensor_tensor(
                            r1_s[:, j, :], to[:], xq_s[:, j, :], OP.add
                        )
                        sq_t = op_.tile([P, SQ], f32r, tag="sqt", bufs=2)
                        nc.vector.tensor_tensor(
                            sq_t[:], r1_s[:, j, :], r1_s[:, j, :], OP.mult
                        )
                        for n in range(2):
                            mm(
                                sum_ps[:, n * 512 : (n + 1) * 512],
                                ones_s[:],
                                r1_s[:, j, n * 512 : (n + 1) * 512],
                                j == 0, j == HC - 1,
                            )
                            mm(
                                sq_ps[:, n * 512 : (n + 1) * 512],
                                ones_s[:],
                                sq_t[:, n * 512 : (n + 1) * 512],
                                j == 0, j == HC - 1,
                            )
                    if DEBUG_DUMPS:
                        nc.sync.dma_start(
                            t["dr1"], r1_s[:].bitcast(f32)
                        )
                    _ln_normalize(
                        nc, tc, mybir, op_, sum_ps, sq_ps, r1_s,
                        [x1_s, x18_s], l1w_s, l1b_s,
                    )
                    if DEBUG_DUMPS:
                        nc.sync.dma_start(t["dx1"], x1_s[:])

        # ---------------- MLP + LN2 + output ----------------
        with tc.tile_pool(name="mlp", bufs=1) as mp, tc.tile_pool(
            name="st2_ps", bufs=1, space="PSUM"
        ) as ppst2:
            hT_s = mp.tile([P, FC, SQ], m2_dt)
            r2_s = mp.tile([P, HC, SQ], bf16)
            # W2 in two halves so MLP2's first chunks don't wait for the
            # whole tensor; both transfer during the LN1 tail + MLP1.
            w2_s = mp.tile([P, FC, H], m2_dt)
            w2_r = t["W2"].rearrange("(c p) m -> p c m", p=P)
            nc.sync.dma_start(w2_s[:, 0 : FC // 2, :], w2_r[:, 0 : FC // 2, :])
            nc.scalar.dma_start(w2_s[:, FC // 2 :, :], w2_r[:, FC // 2 :, :])
            sum2_ps = ppst2.tile([1, SQ], f32, tag="ln2sum", bufs=1)
            sq2_ps = ppst2.tile([1, SQ], f32, tag="ln2sq", bufs=1)
            with tc.tile_pool(name="m_ps", bufs=2, space="PSUM") as ppm:
                for m in range(FC):
                    ps = ppm.tile([P, SQ], f32, tag="mps")
                    proj_accum(
                        ppm, w1all[:, :, m * P : (m + 1) * P], x18_s, 0,
                        ps, FP8_MLP1, [(0, 512), (512, 512)],
                    )
                    nc.scalar.activation(
                        hT_s[:, m, :], ps[:], AF.Gelu,
                        bias=b1_s[:, m : m + 1], scale=m1_sc,
                    )

                for j in range(HC):
                    ps = ppm.tile([P, SQ], f32, tag="mps")
                    if FP8_MLP2:
                        for n in range(2):
                            for kc2 in range(FC // 2):
                                mm(
                                    ps[:, n * 512 : (n + 1) * 512],
                                    w2_s[
                                        :, 2 * kc2 : 2 * kc2 + 2,
                                        j * P : (j + 1) * P,
                                    ],
                                    hT_s[
                                        :, 2 * kc2 : 2 * kc2 + 2,
                                        n * 512 : (n + 1) * 512,
                                    ],
                                    kc2 == 0,
                                    kc2 == FC // 2 - 1,
                                    perf_mode=DR,
                                )
                    else:
                        for n in range(2):
                            for kc in range(FC):
                                mm(
                                    ps[:, n * 512 : (n + 1) * 512],
                                    w2_s[:, kc, j * P : (j + 1) * P],
                                    hT_s[:, kc, n * 512 : (n + 1) * 512],
                                    kc == 0,
                                    kc == FC - 1,
                                )
                    to = mp.tile([P, SQ], f32, tag="to2", bufs=1)
                    nc.scalar.activation(
                        to[:], ps[:], AF.Identity,
                        bias=b2_s[:, j : j + 1], scale=m2_sc,
                    )
                    nc.vector.tensor_tensor(
                        r2_s[:, j, :], to[:], x1_s[:, j, :], OP.add
                    )
                    sq_t = mp.tile([P, SQ], bf16, tag="sqt2", bufs=2)
                    nc.vector.tensor_tensor(
                        sq_t[:], r2_s[:, j, :], r2_s[:, j, :], OP.mult
                    )
                    for n in range(2):
                        mm(
                            sum2_ps[:, n * 512 : (n + 1) * 512],
                            ones_b[:],
                            r2_s[:, j, n * 512 : (n + 1) * 512],
                            j == 0, j == HC - 1,
                        )
                        mm(
                            sq2_ps[:, n * 512 : (n + 1) * 512],
                            ones_b[:],
                            sq_t[:, n * 512 : (n + 1) * 512],
                            j == 0, j == HC - 1,
                        )
                if DEBUG_DUMPS:
                    nc.sync.dma_start(t["dh"], hT_s[:])
                    nc.sync.dma_start(t["dr2"], r2_s[:])

            # LN2 normalizes r2 in place; each chunk is transposed to
            # token-major as soon as it is normalized, and each token tile
            # is stored as soon as its last chunk lands.
            with tc.tile_pool(name="outp", bufs=1) as outp, tc.tile_pool(
                name="tr_ps", bufs=4, space="PSUM"
            ) as ppt:
                out_all = outp.tile([P, TQ, H], bf16)

                def ln2_chunk(j):
                    for tt in range(TQ):
                        tps = ppt.tile([P, P], bf16, tag="tr")
                        nc.tensor.transpose(
                            tps[:],
                            r2_s[:, j, tt * P : (tt + 1) * P],
                            ident_b[:],
                        )
                        if tt % 2 == 0:
                            nc.scalar.activation(
                                out_all[:, tt, j * P : (j + 1) * P], tps[:],
                                AF.Identity,
                            )
                        else:
                            nc.vector.tensor_copy(
                                out_all[:, tt, j * P : (j + 1) * P], tps[:]
                            )

                _ln_normalize(
                    nc, tc, mybir, mp, sum2_ps, sq2_ps, r2_s, [r2_s],
                    l2w_s, l2b_s, chunk_cb=ln2_chunk,
                )
                for tt in range(TQ):
                    qs[tt % 3].dma_start(
                        t["y"][tt * P : (tt + 1) * P, :], out_all[:, tt, :]
                    )


def _ln_normalize(nc, tc, mybir, pool, sum_ps, sq_ps, src_s, dsts, w_s, b_s,
                  chunk_cb=None):
    """Feature-major LayerNorm given accumulated sum / sum-of-squares rows.

    src_s: [P, HC, SQ]. Stats ops run on [1, SQ] rows; mean/rstd broadcast
    across partitions on the GPSIMD engine; the two tensor-tensor steps run
    on the DVE; the final scale-bias (with dtype cast) runs on the Act
    engine once per entry in `dsts` (each a [P, HC, SQ] AP). chunk_cb(j)
    is called after chunk j's outputs are written (for pipelining)."""
    f32 = mybir.dt.float32
    AF = mybir.ActivationFunctionType
    OP = mybir.AluOpType

    f32r = mybir.dt.float32r

    def fm_slice(j):
        sl = src_s[:, j, :]
        return sl.bitcast(f32) if src_s.dtype == f32r else sl

    # Stats chain spread across engines: mean on Act, its broadcast on
    # GPSIMD immediately after, the variance/reciprocal smalls on DVE,
    # rstd sqrt back on Act, its broadcast on GPSIMD. The (src - mean)
    # step for half the chunks runs on GPSIMD between the two
    # broadcasts, in parallel with the DVE smalls.
    mean = pool.tile([1, SQ], f32, tag="lnmean", bufs=1)
    nc.scalar.activation(mean[:], sum_ps[:], AF.Identity, scale=1.0 / H)
    mb = pool.tile([P, SQ], f32, tag="lnmb", bufs=1)
    nc.gpsimd.partition_broadcast(mb[:], mean[:])
    m2 = pool.tile([1, SQ], f32, tag="lnsm", bufs=2)
    nc.vector.tensor_tensor(m2[:], mean[:], mean[:], OP.mult)
    vpe = pool.tile([1, SQ], f32, tag="lnsm", bufs=2)
    nc.vector.scalar_tensor_tensor(
        out=vpe[:], in0=sq_ps[:], scalar=1.0 / H, in1=m2[:], op0=OP.mult,
        op1=OP.subtract,
    )
    nc.vector.tensor_scalar_add(vpe[:], vpe[:], EPS)
    rvar = pool.tile([1, SQ], f32, tag="lnsm", bufs=2)
    with nc.allow_low_precision(reason="18-bit 1/(var+eps) is benign"):
        nc.vector.reciprocal_approx_fast(out=rvar[:], in_=vpe[:])
    rstd = pool.tile([1, SQ], f32, tag="lnrstd", bufs=1)
    nc.scalar.activation(rstd[:], rvar[:], AF.Sqrt)
    rb = pool.tile([P, SQ], f32, tag="lnrb", bufs=1)
    nc.gpsimd.partition_broadcast(rb[:], rstd[:])
    for j in range(HC):
        t1 = pool.tile([P, SQ], f32, tag="lnt1v", bufs=2)
        nc.vector.tensor_tensor(t1[:], fm_slice(j), mb[:], OP.subtract)
        t2 = pool.tile([P, SQ], f32, tag="lnt2", bufs=2)
        nc.vector.tensor_tensor(t2[:], t1[:], rb[:], OP.mult)
        for dst in dsts:
            nc.scalar.activation(
                dst[:, j, :], t2[:], AF.Identity,
                bias=b_s[:, j : j + 1], scale=w_s[:, j : j + 1],
            )
        if chunk_cb is not None:
            chunk_cb(j)


def _build():
    import concourse.bacc as bacc
    import concourse.tile as tile
    import concourse.mybir as mybir
    from concourse.masks import make_identity

    f32 = mybir.dt.float32
    bf16 = mybir.dt.bfloat16
    fp8 = mybir.dt.float8e4

    qkv_dt = fp8 if FP8_QKV else bf16
    o_dt = fp8 if FP8_O else bf16
    m1_dt = fp8 if FP8_MLP1 else bf16
    m2_dt = fp8 if FP8_MLP2 else bf16

    nc = bacc.Bacc(
        "TRN2", target_bir_lowering=False, debug=False, num_devices=N_CORES
    )
    specs = [
        ("xT", [H, S], qkv_dt, "ExternalInput"),
        ("xq", [H, SQ], bf16, "ExternalInput"),
        ("xq8", [H, SQ], qkv_dt, "ExternalInput"),
        ("Wq", [H, H], qkv_dt, "ExternalInput"),
        ("Wk", [H, H], qkv_dt, "ExternalInput"),
        ("Wv", [H, H], qkv_dt, "ExternalInput"),
        ("Wo", [H, H], o_dt, "ExternalInput"),
        ("W1", [H, FF], m1_dt, "ExternalInput"),
        ("W2", [FF, H], m2_dt, "ExternalInput"),
        ("bq2", [P, HC], f32, "ExternalInput"),
        ("bk2", [P, HC], f32, "ExternalInput"),
        ("bv", [H], f32, "ExternalInput"),
        ("bo2", [P, HC], f32, "ExternalInput"),
        ("b12", [P, FC], f32, "ExternalInput"),
        ("b22", [P, HC], f32, "ExternalInput"),
        ("l1w", [P, HC], f32, "ExternalInput"),
        ("l1b", [P, HC], f32, "ExternalInput"),
        ("l2w", [P, HC], f32, "ExternalInput"),
        ("l2b", [P, HC], f32, "ExternalInput"),
        ("y", [SQ, H], bf16, "ExternalOutput"),
    ]
    if DEBUG_DUMPS:
        av_dt = fp8 if FP8_AV else bf16
        vs = 128 if FP8_AV else 65
        vcols = (12 - 1) * vs + P
        specs += [
            ("dq", [P, NH, SQ], bf16, "ExternalOutput"),
            ("dk", [P, HC, S], bf16, "ExternalOutput"),
            ("dv", [P, 16, vcols], av_dt, "ExternalOutput"),
            ("dattn", [P, HC, SQ], o_dt, "ExternalOutput"),
            ("dr1", [P, HC, SQ], f32, "ExternalOutput"),
            ("dx1", [P, HC, SQ], bf16, "ExternalOutput"),
            ("dh", [P, FC, SQ], m2_dt, "ExternalOutput"),
            ("dr2", [P, HC, SQ], bf16, "ExternalOutput"),
        ]
    t = {
        name: nc.dram_tensor(name, shape, dt, kind=kind).ap()
        for name, shape, dt, kind in specs
    }
    with tile.TileContext(nc) as tc:
        _emit(nc, tc, t, mybir, make_identity)
    nc.compile()
    return nc


def _chunk_major(v):
    """[C*P] -> [P, C] with entry [p, c] = v[c*P + p]."""
    return np.ascontiguousarray(v.reshape(-1, P).T)


def prepare_in_maps(inputs):
    inp = {k: np.asarray(v) for k, v in inputs.items()}
    x = inp["x"].astype(np.float32)

    def wcast(w, on):
        w = w.astype(np.float32)
        if on:
            return (w * WSCALE).astype(F8)
        return w.astype(BF16)

    shared = {
        "Wq": wcast(inp["Wq"], FP8_QKV),
        "Wk": wcast(inp["Wk"], FP8_QKV),
        "Wv": wcast(inp["Wv"], FP8_QKV),
        "Wo": wcast(inp["Wo"], FP8_O),
        "W1": wcast(inp["W1"], FP8_MLP1),
        "W2": wcast(inp["W2"], FP8_MLP2),
        "bq2": _chunk_major(inp["bq"].astype(np.float32)),
        "bk2": _chunk_major(inp["bk"].astype(np.float32)),
        "bv": inp["bv"].astype(np.float32),
        "bo2": _chunk_major(inp["bo"].astype(np.float32)),
        "b12": _chunk_major(inp["b1"].astype(np.float32)),
        "b22": _chunk_major(inp["b2"].astype(np.float32)),
        "l1w": _chunk_major(inp["ln1_w"].astype(np.float32)),
        "l1b": _chunk_major(inp["ln1_b"].astype(np.float32)),
        "l2w": _chunk_major(inp["ln2_w"].astype(np.float32)),
        "l2b": _chunk_major(inp["ln2_b"].astype(np.float32)),
    }
    xdt = F8 if FP8_QKV else BF16
    in_maps = []
    for c in range(N_CORES):
        b, hf = c // 2, c % 2
        xT = np.ascontiguousarray(x[b].T)
        m = dict(shared)
        m["xT"] = xT.astype(xdt)
        xqs = np.ascontiguousarray(xT[:, hf * SQ : (hf + 1) * SQ])
        m["xq"] = xqs.astype(BF16)
        m["xq8"] = xqs.astype(xdt)
        in_maps.append(m)
    return in_maps


def get_program():
    if "nc" not in _CACHE:
        _CACHE["nc"] = _build()
    return _CACHE["nc"]


def kernel(**inputs):
    from concourse.bass_utils import run_bass_kernel_spmd

    nc = get_program()
    in_maps = prepare_in_maps(inputs)
    res = run_bass_kernel_spmd(nc, in_maps, core_ids=list(range(N_CORES)))
    out = np.empty((B, S, H), np.float32)
    for c in range(N_CORES):
        b, hf = c // 2, c % 2
        out[b, hf * SQ : (hf + 1) * SQ] = res.results[c]["y"]
    return out
